# revision 1
# baseline (speedup 1.0000x reference)
"""BitGQA attention kernel for 8 trn2 NeuronCores.

Sharding: 8 cores = 2 batch groups x 4 tensor-parallel groups.
Core c handles batch b = c//4 and head-group g = c%4 (8 q heads, 2 kv heads,
512-wide slices of the q/o projections, 128-wide slices of k/v).

Host prep (once per weight set, outside the timed device program):
  - ternary-quantize all four projection weights (exact reference math:
    clip(round(w/mean|w|), -1, 1)) and ship them as one concatenated
    transposed bf16 matrix [2048, 512+128+128+512] per rank
  - ship the 4 w_scales, rope tables in transposed halved layout, and the
    rotate-half permutation matrix

Device dataflow (per core), activations transposed [feature, token] so the
contraction dim sits on partitions:
  1. stream x token tiles: row stats (ACT square-accum + Pool abs-max),
     absmax-quantize via fp32 magic rounding, ONE strided 3D xbar-transpose
     per tile into the column-interleaved xqT layout
  2. q/k/v projections (bf16 matmuls, exact integer x ternary); RoPE applied
     as q*cosf + (P@q)*sinf where P is a permutation matmul and the
     cos/sin tables have the per-token dequant scales folded in
  3. causal attention per head, software-pipelined: rotating PSUM score
     banks so PE score-matmuls run ahead of ACT exp; softmax denominator
     falls out of the same matmul via the [v|1] augmented operand
  4. attention-output normalization in transposed layout, stats via PE
     transpose into one PSUM bank (ACT square-accum + DVE absmax straight
     from PSUM), tiny stats AllGather, absmax-quantize to int8
  5. int8 AllGather of the o-projection input in 2 token-halves overlapped
     with the o-projection matmuls (SWDGE cast-DMAs int8->bf16)
  6. o-projection, per-token rescale, write the [t, 512] slice.

The final output is assembled on host from the 8 [2048, 512] slices.
"""

import contextlib

import numpy as np

import concourse.bass as bass
import concourse.bacc as bacc
import concourse.mybir as mybir
import concourse.tile as tile
from concourse import bass_utils

F32 = mybir.dt.float32
BF16 = mybir.dt.bfloat16
I8 = mybir.dt.int8
AF = mybir.ActivationFunctionType
ALU = mybir.AluOpType

MAGIC = float(1.5 * 2.0**23)  # fp32 round-to-nearest-even magic constant
EPS_NORM = 1e-6
EPS_Q = 1e-5

N_CORES = 8
D = 2048
H_TOTAL, KV_TOTAL, HD = 32, 8, 64
G = 4  # tensor-parallel groups
NH = H_TOTAL // G          # 8 local q heads
NKV = KV_TOTAL // G        # 2 local kv heads
QO = NH * HD               # 512 local q dims
KO = NKV * HD              # 128 local kv dims
ND = D // 128              # 16 d-tiles
WCAT = QO + KO + KO + QO   # 1280 concatenated weight columns
C_Q, C_K, C_V, C_O = 0, QO, QO + KO, QO + 2 * KO


def build_program(T=2048, has_g=False, n_cores=N_CORES,
                  emulate_collectives=False):
    NT = T // 128   # token tiles
    NJ = T // 512   # 512-wide token columns
    NO = QO // 128  # q/o o-tiles (4)
    TH = T // 2     # token half for the chunked o-gather
    NTH = NT // 2
    rg = ([[0, 1, 2, 3], [4, 5, 6, 7]] if n_cores == N_CORES else
          [[c] for c in range(n_cores)])

    nc = bacc.Bacc("TRN2", target_bir_lowering=False, debug=False,
                   num_devices=n_cores)

    # ---- per-core DRAM I/O ----
    x_d = nc.dram_tensor("x", [T, D], F32, kind="ExternalInput")
    wcat_d = nc.dram_tensor("wcat", [D, WCAT], BF16, kind="ExternalInput")
    cost2_d = nc.dram_tensor("cost2", [128, T], BF16, kind="ExternalInput")
    sint2s_d = nc.dram_tensor("sint2s", [128, T], BF16, kind="ExternalInput")
    pswap_d = nc.dram_tensor("pswap", [128, 128], BF16, kind="ExternalInput")
    ws_d = nc.dram_tensor("ws_r", [1, 4], F32, kind="ExternalInput")
    go_d = nc.dram_tensor("go_r", [1, QO], F32, kind="ExternalInput")
    if has_g:
        g_d = nc.dram_tensor("g_r", [1, D], F32, kind="ExternalInput")
    out_d = nc.dram_tensor("out", [T, QO], F32, kind="ExternalOutput")

    with tile.TileContext(nc) as tc, contextlib.ExitStack() as stack:
        # ---------------- long-lived pools (strict stack order) --------------
        singles = stack.enter_context(tc.tile_pool(name="singles", bufs=1))
        cols = stack.enter_context(tc.tile_pool(name="cols", bufs=1))
        dram = stack.enter_context(tc.tile_pool(name="dram", bufs=1,
                                                space="DRAM"))

        # constants
        zero_col = singles.tile([128, 1], F32)
        nc.vector.memset(zero_col, 0.0)
        magic_col = singles.tile([128, 1], F32)
        nc.vector.memset(magic_col, MAGIC)
        epsn_col = singles.tile([128, 1], F32)
        nc.vector.memset(epsn_col, EPS_NORM)

        identity = singles.tile([128, 128], BF16)
        nc.gpsimd.memset(identity, 1.0)
        nc.gpsimd.affine_select(out=identity, in_=identity, compare_op=ALU.is_ge,
                                fill=0.0, base=0, pattern=[[-1, 128]],
                                channel_multiplier=1)
        nc.gpsimd.affine_select(out=identity, in_=identity, compare_op=ALU.is_ge,
                                fill=0.0, base=0, pattern=[[1, 128]],
                                channel_multiplier=-1)

        # causal mask for diagonal 128x128 blocks of scoresT[k, t]:
        # keep 1.0 where t >= k i.e. (free - partition) >= 0
        trimask = singles.tile([128, 128], BF16)
        nc.gpsimd.memset(trimask, 1.0)
        nc.gpsimd.affine_select(out=trimask, in_=trimask, compare_op=ALU.is_ge,
                                fill=0.0, base=0, pattern=[[1, 128]],
                                channel_multiplier=-1)

        pswap = singles.tile([128, 128], BF16)
        nc.sync.dma_start(out=pswap, in_=pswap_d[:, :])

        # go as per-head columns [64, NH]
        go_cols = singles.tile([64, NH], F32)
        nc.sync.dma_start(out=go_cols,
                          in_=go_d[0:1, :].rearrange("1 (h p) -> p h", p=64))

        ws_cols = singles.tile([128, 4], F32)
        nc.sync.dma_start(out=ws_cols, in_=ws_d[0:1, :].to_broadcast((128, 4)))

        if has_g:
            g_bcast = singles.tile([128, D], F32)
            nc.sync.dma_start(out=g_bcast,
                              in_=g_d[0:1, :].to_broadcast((128, D)))

        # ====== lifetime pools, opened in reverse-close order ================
        es_w = contextlib.ExitStack()
        w_pool = es_w.enter_context(tc.tile_pool(name="wp", bufs=1))
        es_ao = contextlib.ExitStack()
        ao_pool = es_ao.enter_context(tc.tile_pool(name="aop", bufs=1))
        es_qkv = contextlib.ExitStack()
        qkv_pool = es_qkv.enter_context(tc.tile_pool(name="qkv", bufs=1))
        es_proj = contextlib.ExitStack()
        proj_pool = es_proj.enter_context(tc.tile_pool(name="proj", bufs=1))

        # quantized weights: 16 tiles [128, 1280] (q|k|v|o column blocks)
        wt = [w_pool.tile([128, WCAT], BF16, name=f"wt{r}") for r in range(ND)]
        for r in range(ND):
            nc.sync.dma_start(out=wt[r], in_=wcat_d[r * 128:(r + 1) * 128, :])

        cost2 = proj_pool.tile([128, T], BF16)
        nc.sync.dma_start(out=cost2, in_=cost2_d[:, :])
        sint2s = proj_pool.tile([128, T], BF16)
        nc.sync.dma_start(out=sint2s, in_=sint2s_d[:, :])

        # attention operands (qkv lifetime)
        qT = [qkv_pool.tile([128, T], BF16, name=f"qT{a}") for a in range(NO)]
        kT = qkv_pool.tile([128, T], BF16)
        v1 = [[qkv_pool.tile([128, HD + 1], BF16, name=f"v1_{kv}_{r}")
               for r in range(NT)] for kv in range(NKV)]
        for kv in range(NKV):
            for r in range(NT):
                nc.vector.memset(v1[kv][r][:, HD:HD + 1], 1.0)

        # x-quant stat columns
        ss_col = cols.tile([128, NT], F32)
        amax_col = cols.tile([128, NT], F32)
        rsq_col = cols.tile([128, NT], F32)
        xsc_col = cols.tile([128, NT], F32)
        s_col = cols.tile([128, NT], F32)
        scr1_col = cols.tile([128, NT], F32)
        fv_col = cols.tile([128, NT], F32)
        xsc_d = dram.tile([1, T], F32)

        # -------- phases X+P merged: stream token columns of 512 -------------
        with tc.tile_pool(name="xpool", bufs=1 if has_g else 2) as xpool, \
             tc.tile_pool(name="xscr", bufs=1) as xscr, \
             tc.tile_pool(name="xqTc", bufs=1 if has_g else 2) as xqTc_pool, \
             tc.tile_pool(name="fqfp", bufs=1) as fqfp, \
             tc.tile_pool(name="rawp", bufs=1 if has_g else 2) as rawp, \
             tc.tile_pool(name="psq", bufs=1, space="PSUM") as psq, \
             tc.tile_pool(name="psk", bufs=1, space="PSUM") as psk, \
             tc.tile_pool(name="psv", bufs=1, space="PSUM") as psv, \
             tc.tile_pool(name="psro", bufs=2, space="PSUM") as psro:
            for j in range(NJ):
                jc = slice(j * 512, (j + 1) * 512)
                # interleaved layout: block r at cols [r*512+s*128, +128)
                xqTc = xqTc_pool.tile([128, ND * 512], BF16, tag="xqTc")
                xqTc_v = xqTc.rearrange("p (r s c) -> p r s c", r=ND, s=4)
                # ---- X: quantize 4 token tiles of this column ----
                cj = slice(4 * j, 4 * j + 4)
                xts = []
                for s4 in range(4):
                    i = 4 * j + s4
                    ci = slice(i, i + 1)
                    xt = xpool.tile([128, D], F32, tag=f"xt{s4}",
                                    name=f"xt{s4}", bufs=1)
                    nc.sync.dma_start(out=xt,
                                      in_=x_d[i * 128:(i + 1) * 128, :])
                    if has_g:
                        xg = xpool.tile([128, D], F32, tag=f"xg{s4}",
                                        name=f"xg{s4}", bufs=1)
                        nc.vector.tensor_tensor(out=xg, in0=xt, in1=g_bcast,
                                                op=ALU.mult)
                        src = xg
                    else:
                        src = xt
                    xts.append(src)
                    # discard target for Square reuses the scratch slot bytes
                    sq_scr = xscr.tile([128, D], F32, tag="xtmp")
                    nc.scalar.activation(sq_scr.bitcast(BF16)[:, 0:D], xt,
                                         AF.Square, bias=zero_col,
                                         scale=1.0, accum_out=ss_col[:, ci])
                    nc.vector.tensor_reduce(out=amax_col[:, ci], in_=src,
                                            axis=mybir.AxisListType.X,
                                            op=ALU.max,
                                            apply_absolute_value=True)
                # batched per-column stat math on [128, 4] slices
                nc.scalar.activation(scr1_col[:, cj], ss_col[:, cj],
                                     AF.Sqrt, bias=epsn_col, scale=1.0 / D)
                nc.vector.reciprocal(rsq_col[:, cj], scr1_col[:, cj])
                nc.vector.tensor_tensor(out=xsc_col[:, cj],
                                        in0=amax_col[:, cj],
                                        in1=rsq_col[:, cj], op=ALU.mult)
                nc.vector.tensor_scalar_max(xsc_col[:, cj], xsc_col[:, cj],
                                            EPS_Q)
                nc.vector.reciprocal(scr1_col[:, cj], xsc_col[:, cj])
                nc.vector.tensor_tensor(out=s_col[:, cj], in0=rsq_col[:, cj],
                                        in1=scr1_col[:, cj], op=ALU.mult)
                nc.vector.tensor_scalar_mul(s_col[:, cj], s_col[:, cj], 127.0)
                nc.vector.tensor_scalar(fv_col[:, cj], xsc_col[:, cj],
                                        ws_cols[:, 2:3], 1.0 / 127.0,
                                        op0=ALU.mult, op1=ALU.mult)
                for s4 in range(4):
                    i = 4 * j + s4
                    tmp = xscr.tile([128, D], F32, tag="xtmp")
                    nc.scalar.activation(tmp, xts[s4], AF.Identity,
                                         bias=magic_col,
                                         scale=s_col[:, i:i + 1])
                    xq = xpool.tile([128, D], BF16, tag="xq")
                    nc.vector.tensor_scalar_sub(xq, tmp, MAGIC)
                    # one xbar transpose for all 16 d-tiles of this token tile
                    nc.sync.dma_start_transpose(
                        out=xqTc_v[:, :, s4, :], in_=xq)

                # ---- per-token dequant factors folded into rope tables ----
                nc.sync.dma_start(
                    out=xsc_d[0:1, jc].rearrange("1 (i p) -> p i", p=128),
                    in_=xsc_col[:, cj])
                fq_f = fqfp.tile([128, 512], F32, tag="fqf")
                nc.sync.dma_start(out=fq_f,
                                  in_=xsc_d[0:1, jc].to_broadcast((128, 512)))
                xf_q = fqfp.tile([128, 512], BF16, tag="xfq")
                xf_k = fqfp.tile([128, 512], BF16, tag="xfk")
                nc.vector.tensor_scalar(xf_q, fq_f, ws_cols[:, 0:1],
                                        1.0 / 127.0, op0=ALU.mult, op1=ALU.mult)
                nc.vector.tensor_scalar(xf_k, fq_f, ws_cols[:, 1:2],
                                        1.0 / (127.0 * float(np.sqrt(HD))),
                                        op0=ALU.mult, op1=ALU.mult)
                cosq = fqfp.tile([128, 512], BF16, tag="cosq")
                sinq = fqfp.tile([128, 512], BF16, tag="sinq")
                cosk = fqfp.tile([128, 512], BF16, tag="cosk")
                sink = fqfp.tile([128, 512], BF16, tag="sink")
                nc.vector.tensor_tensor(out=cosq, in0=cost2[:, jc], in1=xf_q,
                                        op=ALU.mult)
                nc.vector.tensor_tensor(out=sinq, in0=sint2s[:, jc], in1=xf_q,
                                        op=ALU.mult)
                nc.vector.tensor_tensor(out=cosk, in0=cost2[:, jc], in1=xf_k,
                                        op=ALU.mult)
                nc.vector.tensor_tensor(out=sink, in0=sint2s[:, jc], in1=xf_k,
                                        op=ALU.mult)

                # ---- P: projections for this column ----
                ps_q = [psq.tile([128, 512], F32, tag=f"q{a}", name=f"ps_q{a}")
                        for a in range(NO)]
                ps_k = psk.tile([128, 512], F32)
                ps_v = psv.tile([128, 512], F32)
                for r in range(ND):
                    ch = xqTc[:, r * 512:(r + 1) * 512]
                    st = dict(start=(r == 0), stop=(r == ND - 1))
                    for a in range(NO):
                        nc.tensor.matmul(
                            ps_q[a],
                            wt[r][:, C_Q + a * 128:C_Q + (a + 1) * 128],
                            ch, **st)
                    nc.tensor.matmul(ps_k, wt[r][:, C_K:C_K + KO], ch, **st)
                for s in range(4):
                    for r in range(ND):
                        nc.tensor.matmul(
                            ps_v[:, s * 128:(s + 1) * 128],
                            xqTc[:, r * 512 + s * 128:r * 512 + (s + 1) * 128],
                            wt[r][:, C_V:C_V + KO], start=(r == 0),
                            stop=(r == ND - 1))
                for s in range(4):
                    kt_i = 4 * j + s
                    for kv in range(NKV):
                        nc.scalar.activation(
                            v1[kv][kt_i][:, 0:HD],
                            ps_v[:, s * 128 + kv * HD:s * 128 + (kv + 1) * HD],
                            AF.Copy, bias=0.0,
                            scale=fv_col[:, kt_i:kt_i + 1])

                # ---- rope: dst = raw*cosf + (P@raw)*sinf ----
                def rope(dst, ps_raw, cosf, sinf):
                    raw = rawp.tile([128, 512], BF16, tag="raw")
                    nc.scalar.copy(out=raw, in_=ps_raw)
                    ps_sh = psro.tile([128, 512], F32, tag="sh")
                    nc.tensor.matmul(ps_sh, pswap, raw, start=True, stop=True)
                    sh = rawp.tile([128, 512], BF16, tag="sh")
                    nc.vector.tensor_tensor(out=sh, in0=ps_sh, in1=sinf,
                                            op=ALU.mult)
                    cq = rawp.tile([128, 512], BF16, tag="cq")
                    nc.vector.tensor_tensor(out=cq, in0=raw, in1=cosf,
                                            op=ALU.mult)
                    nc.vector.tensor_tensor(out=dst, in0=cq, in1=sh,
                                            op=ALU.add)

                for a in range(NO):
                    rope(qT[a][:, jc], ps_q[a], cosq, sinq)
                rope(kT[:, jc], ps_k, cosk, sink)

        es_proj.close()  # frees rope tables, xqT column tiles

        # kT with kv halves swapped so every q head finds its kv head at its
        # own base partition (matmul requires equal base partitions)
        kT2 = qkv_pool.tile([128, T], BF16)
        nc.vector.tensor_copy(out=kT2[0:64, :], in_=kT[64:128, :])
        nc.vector.tensor_copy(out=kT2[64:128, :], in_=kT[0:64, :])

        ao = [ao_pool.tile([128, T], BF16, name=f"ao{a}") for a in range(NO)]
        sums_d = dram.tile([NH, T], F32)
        rsums_d = dram.tile([NH, T], F32)

        # token chunks for the AO/gather/o-proj pipeline: first chunk spans
        # half the columns, the rest are single columns so each chunk's
        # stats-AllGather + quantize + int8-gather hides under the next
        # chunk's attention compute
        if NJ >= 2:
            chunk_js = [(0, NJ // 2), (NJ // 2, NJ)]
        else:
            chunk_js = [(0, NJ)]
        NCK = len(chunk_js)
        cw = [(je - js) * 512 for js, je in chunk_js]
        W_MAX = max(cw)
        CT_MAX = W_MAX // 128

        # AO stat columns (full-T width, filled per token chunk)
        ss_o_col = cols.tile([128, NT], F32)
        amax_o_col = cols.tile([128, NT], F32)
        so_col = cols.tile([128, NT], F32)
        fo_col = cols.tile([128, NT], F32)
        so_d = dram.tile([1, T], F32)
        xqo_in = [dram.tile([QO, cw[c]], I8, name=f"xqoin{c}")
                  for c in range(NCK)]
        xqo_out = [dram.tile([G * QO, cw[c]], I8, name=f"xqoout{c}")
                   for c in range(NCK)]
        stats_in = [dram.tile([128, cw[c] // 64], F32, name=f"stin{c}")
                    for c in range(NCK)]
        stats_out = [dram.tile([128 * G, cw[c] // 64], F32, name=f"stout{c}")
                     for c in range(NCK)]

        # ------- phases A/AO/O interleaved: attention on token-half 1 -------
        # overlaps stats + quantization + int8 AllGather of token-half 0
        with tc.tile_pool(name="psa", bufs=3, space="PSUM") as psa, \
             tc.tile_pool(name="pso", bufs=2, space="PSUM") as pso, \
             tc.tile_pool(name="pst", bufs=1, space="PSUM") as pst_pool, \
             tc.tile_pool(name="psf", bufs=2, space="PSUM") as psf, \
             tc.tile_pool(name="ptp", bufs=6) as ptp, \
             tc.tile_pool(name="sump", bufs=16) as sump, \
             tc.tile_pool(name="aosc", bufs=2) as aosc, \
             tc.tile_pool(name="rsbp", bufs=2) as rsbp, \
             tc.tile_pool(name="qop", bufs=2) as qop, \
             tc.tile_pool(name="otp", bufs=1) as otp, \
             tc.tile_pool(name="outp", bufs=2) as outp:

            def attn_col(j, h0=0, h1=NH):
                jc = slice(j * 512, (j + 1) * 512)
                nk = 4 * (j + 1)
                for h in range(h0, h1):
                    kv = h // (NH // NKV)
                    a_t, pr = h // 2, (h % 2) * 64
                    qh = qT[a_t][pr:pr + 64, :]
                    ksrc = kT if kv * HD == pr else kT2
                    kh = ksrc[pr:pr + 64, :]
                    ps_o = pso.tile([128, 512], F32, tag="o")
                    ss, pts = {}, {}

                    def emit_s(r):
                        phi = r - 4 * j
                        c0 = 128 * phi if phi > 0 else 0
                        t = psa.tile([128, 512], F32, tag="s")
                        nc.tensor.matmul(
                            t[:, c0:512], kh[:, r * 128:(r + 1) * 128],
                            qh[:, j * 512 + c0:(j + 1) * 512],
                            start=True, stop=True)
                        ss[r] = (t, c0)

                    def emit_exp(r):
                        t, c0 = ss.pop(r)
                        pt = ptp.tile([128, 512], BF16, tag="pt")
                        nc.scalar.activation(pt[:, c0:512], t[:, c0:512],
                                             AF.Exp, bias=zero_col, scale=1.0)
                        if r - 4 * j >= 0:
                            nc.vector.tensor_tensor(
                                out=pt[:, c0:c0 + 128],
                                in0=pt[:, c0:c0 + 128],
                                in1=trimask, op=ALU.mult)
                        pts[r] = (pt, c0)

                    def emit_v(r):
                        pt, c0 = pts.pop(r)
                        # columns < c0 are fully masked: skip them instead of
                        # zero-filling (they were started by earlier k-tiles)
                        nc.tensor.matmul(ps_o[0:HD + 1, c0:512],
                                         v1[kv][r], pt[:, c0:512],
                                         start=(r == 0), stop=(r == nk - 1),
                                         skip_group_check=True)

                    emit_s(0)
                    emit_exp(0)
                    if nk > 1:
                        emit_s(1)
                        emit_exp(1)
                    for r in range(2, nk):
                        emit_s(r)
                        emit_exp(r)
                        emit_v(r - 2)
                    for r in (nk - 2, nk - 1):
                        if r >= 0 and r in pts:
                            emit_v(r)

                    sumstage = sump.tile([1, 512], F32, tag="sumstage")
                    nc.scalar.copy(out=sumstage, in_=ps_o[HD:HD + 1, :])
                    nc.sync.dma_start(out=sums_d[h:h + 1, jc], in_=sumstage)
                    nc.vector.tensor_scalar_mul(ao[a_t][pr:pr + 64, jc],
                                                ps_o[0:HD, :],
                                                go_cols[:, h:h + 1])

            def ao_stats_pre(c):
                # normalize + local stats + stats AllGather launch; no op here
                # waits on a collective, so it can sit anywhere in the queues
                js, je = chunk_js[c]
                W = cw[c]
                CT = W // 128
                th = slice(js * 512, je * 512)
                ih = slice(js * 4, je * 4)
                sums_sb = aosc.tile([NH, W_MAX], F32, tag="sums", name="sums_sb")[:, 0:W]
                nc.sync.dma_start(out=sums_sb, in_=sums_d[:, th])
                nc.vector.reciprocal(sums_sb, sums_sb)
                nc.sync.dma_start(out=rsums_d[:, th], in_=sums_sb)
                # normalize ao in place (transposed layout, per-column rsums)
                for a in range(NO):
                    rsb = rsbp.tile([128, W_MAX], BF16, tag="rsb", name="rsb")[:, 0:W]
                    nc.gpsimd.dma_start(
                        out=rsb[0:64, :],
                        in_=rsums_d[2 * a:2 * a + 1, th].to_broadcast(
                            (64, W)))
                    nc.gpsimd.dma_start(
                        out=rsb[64:128, :],
                        in_=rsums_d[2 * a + 1:2 * a + 2, th].to_broadcast(
                            (64, W)))
                    nc.vector.tensor_tensor(out=ao[a][:, th],
                                            in0=ao[a][:, th],
                                            in1=rsb, op=ALU.mult)
                # per-token stats over the local 512 dims via PE transpose
                for i in range(js * 4, je * 4):
                    ci = slice(i, i + 1)
                    pst = pst_pool.tile([128, 512], BF16, tag="pst")
                    for a in range(NO):
                        nc.tensor.transpose(pst[:, a * 128:(a + 1) * 128],
                                            ao[a][:, i * 128:(i + 1) * 128],
                                            identity)
                    sq_scr = aosc.tile([128, 512], BF16, tag="aosq")
                    nc.scalar.activation(sq_scr, pst, AF.Square,
                                         bias=zero_col, scale=1.0,
                                         accum_out=ss_o_col[:, ci])
                    nc.vector.tensor_reduce(out=amax_o_col[:, ci], in_=pst,
                                            axis=mybir.AxisListType.X,
                                            op=ALU.max,
                                            apply_absolute_value=True)
                # pack partial stats, AllGather to [512, 2*CT]
                stats_sb = aosc.tile([128, 2 * CT_MAX], F32, tag="spack",
                                     name="spack")[:, 0:2 * CT]
                nc.vector.tensor_copy(out=stats_sb[:, 0:CT],
                                      in_=ss_o_col[:, ih])
                nc.vector.tensor_copy(out=stats_sb[:, CT:2 * CT],
                                      in_=amax_o_col[:, ih])
                nc.sync.dma_start(out=stats_in[c][:], in_=stats_sb)
                if emulate_collectives:
                    for p in range(G):
                        nc.sync.dma_start(
                            out=stats_out[c][p * 128:(p + 1) * 128, :],
                            in_=stats_in[c][:])
                else:
                    nc.gpsimd.collective_compute("AllGather", ALU.bypass,
                                                 replica_groups=rg,
                                                 ins=[stats_in[c].opt()],
                                                 outs=[stats_out[c].opt()])

            def ao_stats_post(c):
                # combine gathered stats + quant-scale math; the head of this
                # chain waits on the stats AllGather, so it is emitted half an
                # attention column after ao_stats_pre(c)
                js, je = chunk_js[c]
                W = cw[c]
                CT = W // 128
                th = slice(js * 512, je * 512)
                ih = slice(js * 4, je * 4)
                parts = [aosc.tile([128, 2 * CT_MAX], F32, tag=f"parts{p}",
                                   name=f"parts{p}")[:, 0:2 * CT]
                         for p in range(G)]
                for p in range(G):
                    nc.sync.dma_start(
                        out=parts[p],
                        in_=stats_out[c][p * 128:(p + 1) * 128, :])
                for p in range(1, G):
                    nc.vector.tensor_tensor(out=parts[0][:, 0:CT],
                                            in0=parts[0][:, 0:CT],
                                            in1=parts[p][:, 0:CT],
                                            op=ALU.add)
                    nc.vector.tensor_tensor(out=parts[0][:, CT:2 * CT],
                                            in0=parts[0][:, CT:2 * CT],
                                            in1=parts[p][:, CT:2 * CT],
                                            op=ALU.max)
                ss_full = parts[0][:, 0:CT]
                amax_full = parts[0][:, CT:2 * CT]
                rsq_o = aosc.tile([128, CT_MAX], F32, tag="rsqo", name="rsq_o")[:, 0:CT]
                xsc_o = aosc.tile([128, CT_MAX], F32, tag="xsco", name="xsc_o")[:, 0:CT]
                scr2 = aosc.tile([128, CT_MAX], F32, tag="scr2", name="scr2")[:, 0:CT]
                nc.scalar.activation(scr2, ss_full, AF.Sqrt, bias=epsn_col,
                                     scale=1.0 / (H_TOTAL * HD))
                nc.vector.reciprocal(rsq_o, scr2)
                nc.vector.tensor_tensor(out=xsc_o, in0=amax_full, in1=rsq_o,
                                        op=ALU.mult)
                nc.vector.tensor_scalar_max(xsc_o, xsc_o, EPS_Q)
                nc.vector.reciprocal(scr2, xsc_o)
                nc.vector.tensor_tensor(out=so_col[:, ih], in0=rsq_o,
                                        in1=scr2, op=ALU.mult)
                nc.vector.tensor_scalar_mul(so_col[:, ih], so_col[:, ih],
                                            127.0)
                nc.vector.tensor_scalar(fo_col[:, ih], xsc_o,
                                        ws_cols[:, 3:4], 1.0 / 127.0,
                                        op0=ALU.mult, op1=ALU.mult)
                nc.sync.dma_start(
                    out=so_d[0:1, th].rearrange("1 (i p) -> p i", p=128),
                    in_=so_col[:, ih])

            def ao_quant_gather(c):
                js, je = chunk_js[c]
                W = cw[c]
                th = slice(js * 512, je * 512)
                sob = rsbp.tile([128, W_MAX], BF16, tag="sob", name="sob")[:, 0:W]
                nc.gpsimd.dma_start(out=sob,
                                    in_=so_d[0:1, th].to_broadcast((128, W)))
                for a in range(NO):
                    tmp = qop.tile([128, W_MAX], F32, tag="qtmp", name="qtmp")[:, 0:W]
                    nc.vector.tensor_tensor(out=tmp, in0=ao[a][:, th],
                                            in1=sob, op=ALU.mult)
                    xqo = qop.tile([128, W_MAX], BF16, tag="xqo", name="xqo")[:, 0:W]
                    nc.vector.tensor_scalar(xqo, tmp, MAGIC, MAGIC,
                                            op0=ALU.add, op1=ALU.subtract)
                    xqo8 = qop.tile([128, W_MAX], I8, tag="xqo8", name="xqo8")[:, 0:W]
                    nc.vector.tensor_copy(out=xqo8, in_=xqo)
                    nc.sync.dma_start(
                        out=xqo_in[c][a * 128:(a + 1) * 128, :], in_=xqo8)
                if emulate_collectives:
                    for p in range(G):
                        nc.sync.dma_start(
                            out=xqo_out[c][p * QO:(p + 1) * QO, :],
                            in_=xqo_in[c][:])
                else:
                    nc.gpsimd.collective_compute("AllGather", ALU.bypass,
                                                 replica_groups=rg,
                                                 ins=[xqo_in[c].opt()],
                                                 outs=[xqo_out[c].opt()])

            def oproj(c):
                js, je = chunk_js[c]
                ot = [otp.tile([128, W_MAX], BF16, tag=f"ot{r}",
                               name=f"ot{r}")[:, 0:cw[c]]
                      for r in range(ND)]
                for r in range(ND):
                    # SWDGE cast-DMA int8 -> bf16, bypasses HWDGE
                    nc.gpsimd.dma_start(
                        out=ot[r], in_=xqo_out[c][r * 128:(r + 1) * 128, :])
                for il in range(cw[c] // 128):
                    i = js * 4 + il
                    ps_f = psf.tile([128, 512], F32, tag="f")
                    for r in range(ND):
                        nc.tensor.matmul(ps_f,
                                         ot[r][:, il * 128:(il + 1) * 128],
                                         wt[r][:, C_O:C_O + QO],
                                         start=(r == 0), stop=(r == ND - 1))
                    out_t = outp.tile([128, QO], F32, tag="out")
                    nc.scalar.activation(out_t, ps_f, AF.Copy, bias=0.0,
                                         scale=fo_col[:, i:i + 1])
                    nc.sync.dma_start(out=out_d[i * 128:(i + 1) * 128, :],
                                      in_=out_t)

            for j in range(chunk_js[0][0], chunk_js[0][1]):
                attn_col(j)
            for c in range(NCK):
                ao_stats_pre(c)
                if c + 1 < NCK:
                    js, je = chunk_js[c + 1]
                    # split the next chunk's first column by heads so the
                    # in-order queues reach post/quant only after the stats
                    # AllGather has had half a column of attention to finish
                    attn_col(js, 0, NH // 2)
                    ao_stats_post(c)
                    ao_quant_gather(c)
                    attn_col(js, NH // 2, NH)
                    for j in range(js + 1, je):
                        attn_col(j)
                else:
                    # scheduling-time floors keep the gather-gated
                    # o-projections out of the attention engine queues
                    for cc in range(NCK - 1):
                        with tc.tile_wait_until(1.0 + 0.25 * cc):
                            oproj(cc)
                    ao_stats_post(c)
                    ao_quant_gather(c)
                    with tc.tile_wait_until(1.0 + 0.25 * (NCK - 1)):
                        oproj(c)

        es_qkv.close()  # frees qT, kT, kT2, v1
        es_ao.close()   # frees ao
        es_w.close()

    nc.compile()
    return nc


# ---------------------------------------------------------------------------
# host wrapper
# ---------------------------------------------------------------------------
_CACHE = {}


def _get_program(T, has_g):
    key = (T, has_g)
    if key not in _CACHE:
        _CACHE[key] = build_program(T=T, has_g=has_g)
    return _CACHE[key]


def _tern(w):
    ws = max(np.mean(np.abs(w), dtype=np.float32), np.float32(EPS_Q))
    wq = np.clip(np.rint(w / ws), -1.0, 1.0).astype(np.float32)
    return wq, float(ws)


def make_in_maps(x, cos, sin, wq, wk, wv, wo, gq, gk, gv, go, T):
    import ml_dtypes
    BF = ml_dtypes.bfloat16
    cosT = np.ascontiguousarray(cos.T.astype(np.float32))      # [64, T]
    sinT = np.ascontiguousarray(sin.T.astype(np.float32))
    cost2 = np.concatenate([cosT, cosT], axis=0)               # [128, T]
    sint_signed = np.concatenate([-sinT[0:32], sinT[32:64]], axis=0)
    sint2s = np.concatenate([sint_signed, sint_signed], axis=0)
    cost2 = cost2.astype(BF)
    sint2s = sint2s.astype(BF)

    # rotate-half permutation (pure swap; sign lives in sint2s)
    pswap = np.zeros((128, 128), np.float32)
    for p in range(128):
        blk, off = (p // 64) * 64, p % 64
        pswap[p, blk + (off + 32) % 64] = 1.0
    pswap = pswap.astype(BF)

    wq_t, wsq = _tern(np.asarray(wq, np.float32))
    wk_t, wsk = _tern(np.asarray(wk, np.float32))
    wv_t, wsv = _tern(np.asarray(wv, np.float32))
    wo_t, wso = _tern(np.asarray(wo, np.float32))
    ws_r = np.array([[wsq, wsk, wsv, wso]], np.float32)

    ones = np.ones((D,), np.float32)
    has_g = not (np.array_equal(gq, ones) and np.array_equal(gk, ones)
                 and np.array_equal(gv, ones))
    if has_g:
        assert np.array_equal(gq, gk) and np.array_equal(gk, gv), \
            "per-projection norm weights must match"

    in_maps = []
    for c in range(N_CORES):
        b, g = c // G, c % G
        wcat = np.concatenate([
            wq_t[g * QO:(g + 1) * QO, :].T,
            wk_t[g * KO:(g + 1) * KO, :].T,
            wv_t[g * KO:(g + 1) * KO, :].T,
            wo_t[g * QO:(g + 1) * QO, :].T,
        ], axis=1).astype(BF)
        m = {
            "x": np.ascontiguousarray(x[b].astype(np.float32)),
            "wcat": np.ascontiguousarray(wcat),
            "cost2": cost2,
            "sint2s": sint2s,
            "pswap": pswap,
            "ws_r": ws_r,
            "go_r": np.ascontiguousarray(go[g * QO:(g + 1) * QO][None, :]),
        }
        if has_g:
            m["g_r"] = np.ascontiguousarray(gq[None, :])
        in_maps.append(m)
    return in_maps, has_g


def kernel(x, cos, sin, wq, wk, wv, wo, gq, gk, gv, go):
    x = np.asarray(x, np.float32)
    T = x.shape[1]
    in_maps, has_g = make_in_maps(x, cos, sin, np.asarray(wq, np.float32),
                                  np.asarray(wk, np.float32),
                                  np.asarray(wv, np.float32),
                                  np.asarray(wo, np.float32),
                                  np.asarray(gq, np.float32),
                                  np.asarray(gk, np.float32),
                                  np.asarray(gv, np.float32),
                                  np.asarray(go, np.float32), T)
    nc = _get_program(T, has_g)
    res = bass_utils.run_bass_kernel_spmd(nc, in_maps,
                                          core_ids=list(range(N_CORES)))
    out = np.empty((2, T, D), np.float32)
    for c in range(N_CORES):
        b, g = c // G, c % G
        out[b][:, g * QO:(g + 1) * QO] = res.results[c]["out"]
    return out



# revision 2
# speedup vs baseline: 77.1117x; 77.1117x over previous
"""BitGQA attention kernel for 8 trn2 NeuronCores.

Sharding: 8 cores = 2 batch groups x 4 tensor-parallel groups.
Core c handles batch b = c//4 and head-group g = c%4 (8 q heads, 2 kv heads,
512-wide slices of the q/o projections, 128-wide slices of k/v).

Host prep (once per weight set, outside the timed device program):
  - ternary-quantize all four projection weights (exact reference math:
    clip(round(w/mean|w|), -1, 1)) and ship them as one concatenated
    transposed bf16 matrix [2048, 512+128+128+512] per rank
  - ship the 4 w_scales, rope tables in transposed halved layout, and the
    rotate-half permutation matrix

Device dataflow (per core), activations transposed [feature, token] so the
contraction dim sits on partitions:
  1. stream x token tiles: row stats (ACT square-accum + Pool abs-max),
     absmax-quantize via fp32 magic rounding, ONE strided 3D xbar-transpose
     per tile into the column-interleaved xqT layout
  2. q/k/v projections (bf16 matmuls, exact integer x ternary); RoPE applied
     as q*cosf + (P@q)*sinf where P is a permutation matmul and the
     cos/sin tables have the per-token dequant scales folded in
  3. causal attention per head, software-pipelined: rotating PSUM score
     banks so PE score-matmuls run ahead of ACT exp; softmax denominator
     falls out of the same matmul via the [v|1] augmented operand
  4. attention-output normalization in transposed layout, stats via PE
     transpose into one PSUM bank (ACT square-accum + DVE absmax straight
     from PSUM), tiny stats AllGather, absmax-quantize to int8
  5. int8 AllGather of the o-projection input in 2 token-halves overlapped
     with the o-projection matmuls (SWDGE cast-DMAs int8->bf16)
  6. o-projection, per-token rescale, write the [t, 512] slice.

The final output is assembled on host from the 8 [2048, 512] slices.
"""

import contextlib

import numpy as np

import concourse.bass as bass
import concourse.bacc as bacc
import concourse.mybir as mybir
import concourse.tile as tile
from concourse import bass_utils

F32 = mybir.dt.float32
BF16 = mybir.dt.bfloat16
I8 = mybir.dt.int8
AF = mybir.ActivationFunctionType
ALU = mybir.AluOpType

MAGIC = float(1.5 * 2.0**23)  # fp32 round-to-nearest-even magic constant
EPS_NORM = 1e-6
EPS_Q = 1e-5

N_CORES = 8
D = 2048
H_TOTAL, KV_TOTAL, HD = 32, 8, 64
G = 4  # tensor-parallel groups
NH = H_TOTAL // G          # 8 local q heads
NKV = KV_TOTAL // G        # 2 local kv heads
QO = NH * HD               # 512 local q dims
KO = NKV * HD              # 128 local kv dims
ND = D // 128              # 16 d-tiles
WCAT = QO + KO + KO + QO   # 1280 concatenated weight columns
C_Q, C_K, C_V, C_O = 0, QO, QO + KO, QO + 2 * KO


def build_program(T=2048, has_g=False, n_cores=N_CORES,
                  emulate_collectives=False):
    NT = T // 128   # token tiles
    NJ = T // 512   # 512-wide token columns
    NO = QO // 128  # q/o o-tiles (4)
    TH = T // 2     # token half for the chunked o-gather
    NTH = NT // 2
    rg = ([[0, 1, 2, 3], [4, 5, 6, 7]] if n_cores == N_CORES else
          [[c] for c in range(n_cores)])

    nc = bacc.Bacc("TRN2", target_bir_lowering=False, debug=False,
                   num_devices=n_cores)

    # ---- per-core DRAM I/O ----
    x_d = nc.dram_tensor("x", [T, D], F32, kind="ExternalInput")
    wcat_d = nc.dram_tensor("wcat", [D, WCAT], BF16, kind="ExternalInput")
    cost2_d = nc.dram_tensor("cost2", [128, T], BF16, kind="ExternalInput")
    sint2s_d = nc.dram_tensor("sint2s", [128, T], BF16, kind="ExternalInput")
    pswap_d = nc.dram_tensor("pswap", [128, 128], BF16, kind="ExternalInput")
    ws_d = nc.dram_tensor("ws_r", [1, 4], F32, kind="ExternalInput")
    go_d = nc.dram_tensor("go_r", [1, QO], F32, kind="ExternalInput")
    if has_g:
        g_d = nc.dram_tensor("g_r", [1, D], F32, kind="ExternalInput")
    out_d = nc.dram_tensor("out", [T, QO], F32, kind="ExternalOutput")

    with tile.TileContext(nc) as tc, contextlib.ExitStack() as stack:
        # ---------------- long-lived pools (strict stack order) --------------
        singles = stack.enter_context(tc.tile_pool(name="singles", bufs=1))
        cols = stack.enter_context(tc.tile_pool(name="cols", bufs=1))
        dram = stack.enter_context(tc.tile_pool(name="dram", bufs=1,
                                                space="DRAM"))

        # constants
        zero_col = singles.tile([128, 1], F32)
        nc.vector.memset(zero_col, 0.0)
        magic_col = singles.tile([128, 1], F32)
        nc.vector.memset(magic_col, MAGIC)
        epsn_col = singles.tile([128, 1], F32)
        nc.vector.memset(epsn_col, EPS_NORM)

        identity = singles.tile([128, 128], BF16)
        nc.gpsimd.memset(identity, 1.0)
        nc.gpsimd.affine_select(out=identity, in_=identity, compare_op=ALU.is_ge,
                                fill=0.0, base=0, pattern=[[-1, 128]],
                                channel_multiplier=1)
        nc.gpsimd.affine_select(out=identity, in_=identity, compare_op=ALU.is_ge,
                                fill=0.0, base=0, pattern=[[1, 128]],
                                channel_multiplier=-1)

        # causal mask for diagonal 128x128 blocks of scoresT[k, t]:
        # keep 1.0 where t >= k i.e. (free - partition) >= 0
        trimask = singles.tile([128, 128], BF16)
        nc.gpsimd.memset(trimask, 1.0)
        nc.gpsimd.affine_select(out=trimask, in_=trimask, compare_op=ALU.is_ge,
                                fill=0.0, base=0, pattern=[[1, 128]],
                                channel_multiplier=-1)

        pswap = singles.tile([128, 128], BF16)
        nc.sync.dma_start(out=pswap, in_=pswap_d[:, :])

        # go as per-head columns [64, NH]
        go_cols = singles.tile([64, NH], F32)
        nc.sync.dma_start(out=go_cols,
                          in_=go_d[0:1, :].rearrange("1 (h p) -> p h", p=64))

        ws_cols = singles.tile([128, 4], F32)
        nc.sync.dma_start(out=ws_cols, in_=ws_d[0:1, :].to_broadcast((128, 4)))

        if has_g:
            g_bcast = singles.tile([128, D], F32)
            nc.sync.dma_start(out=g_bcast,
                              in_=g_d[0:1, :].to_broadcast((128, D)))

        # ====== lifetime pools, opened in reverse-close order ================
        es_w = contextlib.ExitStack()
        w_pool = es_w.enter_context(tc.tile_pool(name="wp", bufs=1))
        es_ao = contextlib.ExitStack()
        ao_pool = es_ao.enter_context(tc.tile_pool(name="aop", bufs=1))
        es_qkv = contextlib.ExitStack()
        qkv_pool = es_qkv.enter_context(tc.tile_pool(name="qkv", bufs=1))
        es_proj = contextlib.ExitStack()
        proj_pool = es_proj.enter_context(tc.tile_pool(name="proj", bufs=1))

        # quantized weights: 16 tiles [128, 1280] (q|k|v|o column blocks)
        wt = [w_pool.tile([128, WCAT], BF16, name=f"wt{r}") for r in range(ND)]
        for r in range(ND):
            nc.sync.dma_start(out=wt[r], in_=wcat_d[r * 128:(r + 1) * 128, :])

        cost2 = proj_pool.tile([128, T], BF16)
        nc.sync.dma_start(out=cost2, in_=cost2_d[:, :])
        sint2s = proj_pool.tile([128, T], BF16)
        nc.sync.dma_start(out=sint2s, in_=sint2s_d[:, :])

        # attention operands (qkv lifetime)
        qT = [qkv_pool.tile([128, T], BF16, name=f"qT{a}") for a in range(NO)]
        kT = qkv_pool.tile([128, T], BF16)
        v1 = [[qkv_pool.tile([128, HD + 1], BF16, name=f"v1_{kv}_{r}")
               for r in range(NT)] for kv in range(NKV)]
        for kv in range(NKV):
            for r in range(NT):
                nc.vector.memset(v1[kv][r][:, HD:HD + 1], 1.0)

        # x-quant stat columns
        ss_col = cols.tile([128, NT], F32)
        amax_col = cols.tile([128, NT], F32)
        rsq_col = cols.tile([128, NT], F32)
        xsc_col = cols.tile([128, NT], F32)
        s_col = cols.tile([128, NT], F32)
        scr1_col = cols.tile([128, NT], F32)
        fv_col = cols.tile([128, NT], F32)
        xsc_d = dram.tile([1, T], F32)

        # -------- phases X+P merged: stream token columns of 512 -------------
        with tc.tile_pool(name="xpool", bufs=1 if has_g else 2) as xpool, \
             tc.tile_pool(name="xscr", bufs=1) as xscr, \
             tc.tile_pool(name="xqTc", bufs=1 if has_g else 2) as xqTc_pool, \
             tc.tile_pool(name="fqfp", bufs=1) as fqfp, \
             tc.tile_pool(name="rawp", bufs=1 if has_g else 2) as rawp, \
             tc.tile_pool(name="psq", bufs=1, space="PSUM") as psq, \
             tc.tile_pool(name="psk", bufs=1, space="PSUM") as psk, \
             tc.tile_pool(name="psv", bufs=1, space="PSUM") as psv, \
             tc.tile_pool(name="psro", bufs=2, space="PSUM") as psro:
            for j in range(NJ):
                jc = slice(j * 512, (j + 1) * 512)
                # interleaved layout: block r at cols [r*512+s*128, +128)
                xqTc = xqTc_pool.tile([128, ND * 512], BF16, tag="xqTc")
                xqTc_v = xqTc.rearrange("p (r s c) -> p r s c", r=ND, s=4)
                # ---- X: quantize 4 token tiles of this column ----
                cj = slice(4 * j, 4 * j + 4)
                xts = []
                for s4 in range(4):
                    i = 4 * j + s4
                    ci = slice(i, i + 1)
                    xt = xpool.tile([128, D], F32, tag=f"xt{s4}",
                                    name=f"xt{s4}", bufs=1)
                    nc.sync.dma_start(out=xt,
                                      in_=x_d[i * 128:(i + 1) * 128, :])
                    if has_g:
                        xg = xpool.tile([128, D], F32, tag=f"xg{s4}",
                                        name=f"xg{s4}", bufs=1)
                        nc.vector.tensor_tensor(out=xg, in0=xt, in1=g_bcast,
                                                op=ALU.mult)
                        src = xg
                    else:
                        src = xt
                    xts.append(src)
                    # discard target for Square reuses the scratch slot bytes
                    sq_scr = xscr.tile([128, D], F32, tag="xtmp")
                    nc.scalar.activation(sq_scr.bitcast(BF16)[:, 0:D], xt,
                                         AF.Square, bias=zero_col,
                                         scale=1.0, accum_out=ss_col[:, ci])
                    nc.vector.tensor_reduce(out=amax_col[:, ci], in_=src,
                                            axis=mybir.AxisListType.X,
                                            op=ALU.max,
                                            apply_absolute_value=True)
                # batched per-column stat math on [128, 4] slices
                nc.scalar.activation(scr1_col[:, cj], ss_col[:, cj],
                                     AF.Sqrt, bias=epsn_col, scale=1.0 / D)
                nc.vector.reciprocal(rsq_col[:, cj], scr1_col[:, cj])
                nc.vector.tensor_tensor(out=xsc_col[:, cj],
                                        in0=amax_col[:, cj],
                                        in1=rsq_col[:, cj], op=ALU.mult)
                nc.vector.tensor_scalar_max(xsc_col[:, cj], xsc_col[:, cj],
                                            EPS_Q)
                nc.vector.reciprocal(scr1_col[:, cj], xsc_col[:, cj])
                nc.vector.tensor_tensor(out=s_col[:, cj], in0=rsq_col[:, cj],
                                        in1=scr1_col[:, cj], op=ALU.mult)
                nc.vector.tensor_scalar_mul(s_col[:, cj], s_col[:, cj], 127.0)
                nc.vector.tensor_scalar(fv_col[:, cj], xsc_col[:, cj],
                                        ws_cols[:, 2:3], 1.0 / 127.0,
                                        op0=ALU.mult, op1=ALU.mult)
                for s4 in range(4):
                    i = 4 * j + s4
                    tmp = xscr.tile([128, D], F32, tag="xtmp")
                    nc.scalar.activation(tmp, xts[s4], AF.Identity,
                                         bias=magic_col,
                                         scale=s_col[:, i:i + 1])
                    xq = xpool.tile([128, D], BF16, tag="xq")
                    nc.vector.tensor_scalar_sub(xq, tmp, MAGIC)
                    # one xbar transpose for all 16 d-tiles of this token tile
                    nc.sync.dma_start_transpose(
                        out=xqTc_v[:, :, s4, :], in_=xq)

                # ---- per-token dequant factors folded into rope tables ----
                nc.sync.dma_start(
                    out=xsc_d[0:1, jc].rearrange("1 (i p) -> p i", p=128),
                    in_=xsc_col[:, cj])
                fq_f = fqfp.tile([128, 512], F32, tag="fqf")
                nc.sync.dma_start(out=fq_f,
                                  in_=xsc_d[0:1, jc].to_broadcast((128, 512)))
                xf_q = fqfp.tile([128, 512], BF16, tag="xfq")
                xf_k = fqfp.tile([128, 512], BF16, tag="xfk")
                nc.vector.tensor_scalar(xf_q, fq_f, ws_cols[:, 0:1],
                                        1.0 / 127.0, op0=ALU.mult, op1=ALU.mult)
                nc.vector.tensor_scalar(xf_k, fq_f, ws_cols[:, 1:2],
                                        1.0 / (127.0 * float(np.sqrt(HD))),
                                        op0=ALU.mult, op1=ALU.mult)
                cosq = fqfp.tile([128, 512], BF16, tag="cosq")
                sinq = fqfp.tile([128, 512], BF16, tag="sinq")
                cosk = fqfp.tile([128, 512], BF16, tag="cosk")
                sink = fqfp.tile([128, 512], BF16, tag="sink")
                nc.vector.tensor_tensor(out=cosq, in0=cost2[:, jc], in1=xf_q,
                                        op=ALU.mult)
                nc.vector.tensor_tensor(out=sinq, in0=sint2s[:, jc], in1=xf_q,
                                        op=ALU.mult)
                nc.vector.tensor_tensor(out=cosk, in0=cost2[:, jc], in1=xf_k,
                                        op=ALU.mult)
                nc.vector.tensor_tensor(out=sink, in0=sint2s[:, jc], in1=xf_k,
                                        op=ALU.mult)

                # ---- P: projections for this column ----
                ps_q = [psq.tile([128, 512], F32, tag=f"q{a}", name=f"ps_q{a}")
                        for a in range(NO)]
                ps_k = psk.tile([128, 512], F32)
                ps_v = psv.tile([128, 512], F32)
                for r in range(ND):
                    ch = xqTc[:, r * 512:(r + 1) * 512]
                    st = dict(start=(r == 0), stop=(r == ND - 1))
                    for a in range(NO):
                        nc.tensor.matmul(
                            ps_q[a],
                            wt[r][:, C_Q + a * 128:C_Q + (a + 1) * 128],
                            ch, **st)
                    nc.tensor.matmul(ps_k, wt[r][:, C_K:C_K + KO], ch, **st)
                for s in range(4):
                    for r in range(ND):
                        nc.tensor.matmul(
                            ps_v[:, s * 128:(s + 1) * 128],
                            xqTc[:, r * 512 + s * 128:r * 512 + (s + 1) * 128],
                            wt[r][:, C_V:C_V + KO], start=(r == 0),
                            stop=(r == ND - 1))
                for s in range(4):
                    kt_i = 4 * j + s
                    for kv in range(NKV):
                        nc.scalar.activation(
                            v1[kv][kt_i][:, 0:HD],
                            ps_v[:, s * 128 + kv * HD:s * 128 + (kv + 1) * HD],
                            AF.Copy, bias=0.0,
                            scale=fv_col[:, kt_i:kt_i + 1])

                # ---- rope: dst = raw*cosf + (P@raw)*sinf ----
                def rope(dst, ps_raw, cosf, sinf):
                    raw = rawp.tile([128, 512], BF16, tag="raw")
                    nc.scalar.copy(out=raw, in_=ps_raw)
                    ps_sh = psro.tile([128, 512], F32, tag="sh")
                    nc.tensor.matmul(ps_sh, pswap, raw, start=True, stop=True)
                    sh = rawp.tile([128, 512], BF16, tag="sh")
                    nc.vector.tensor_tensor(out=sh, in0=ps_sh, in1=sinf,
                                            op=ALU.mult)
                    cq = rawp.tile([128, 512], BF16, tag="cq")
                    nc.vector.tensor_tensor(out=cq, in0=raw, in1=cosf,
                                            op=ALU.mult)
                    nc.vector.tensor_tensor(out=dst, in0=cq, in1=sh,
                                            op=ALU.add)

                for a in range(NO):
                    rope(qT[a][:, jc], ps_q[a], cosq, sinq)
                rope(kT[:, jc], ps_k, cosk, sink)

        es_proj.close()  # frees rope tables, xqT column tiles

        # kT with kv halves swapped so every q head finds its kv head at its
        # own base partition (matmul requires equal base partitions)
        kT2 = qkv_pool.tile([128, T], BF16)
        nc.vector.tensor_copy(out=kT2[0:64, :], in_=kT[64:128, :])
        nc.vector.tensor_copy(out=kT2[64:128, :], in_=kT[0:64, :])

        ao = [ao_pool.tile([128, T], BF16, name=f"ao{a}") for a in range(NO)]
        sums_d = dram.tile([NH, T], F32)
        rsums_d = dram.tile([NH, T], F32)

        # token chunks for the AO/gather/o-proj pipeline: first chunk spans
        # half the columns, the rest are single columns so each chunk's
        # stats-AllGather + quantize + int8-gather hides under the next
        # chunk's attention compute
        if NJ >= 2:
            chunk_js = [(0, NJ // 2), (NJ // 2, NJ)]
        else:
            chunk_js = [(0, NJ)]
        NCK = len(chunk_js)
        cw = [(je - js) * 512 for js, je in chunk_js]
        W_MAX = max(cw)
        CT_MAX = W_MAX // 128

        # AO stat columns (full-T width, filled per token chunk)
        ss_o_col = cols.tile([128, NT], F32)
        amax_o_col = cols.tile([128, NT], F32)
        so_col = cols.tile([128, NT], F32)
        fo_col = cols.tile([128, NT], F32)
        so_d = dram.tile([1, T], F32)
        xqo_in = [dram.tile([QO, cw[c]], I8, name=f"xqoin{c}")
                  for c in range(NCK)]
        xqo_out = [dram.tile([G * QO, cw[c]], I8, name=f"xqoout{c}")
                   for c in range(NCK)]
        stats_in = [dram.tile([128, cw[c] // 64], F32, name=f"stin{c}")
                    for c in range(NCK)]
        stats_out = [dram.tile([128 * G, cw[c] // 64], F32, name=f"stout{c}")
                     for c in range(NCK)]

        # ------- phases A/AO/O interleaved: attention on token-half 1 -------
        # overlaps stats + quantization + int8 AllGather of token-half 0
        with tc.tile_pool(name="psa", bufs=3, space="PSUM") as psa, \
             tc.tile_pool(name="pso", bufs=2, space="PSUM") as pso, \
             tc.tile_pool(name="pst", bufs=1, space="PSUM") as pst_pool, \
             tc.tile_pool(name="psf", bufs=2, space="PSUM") as psf, \
             tc.tile_pool(name="ptp", bufs=6) as ptp, \
             tc.tile_pool(name="sump", bufs=16) as sump, \
             tc.tile_pool(name="aosc", bufs=2) as aosc, \
             tc.tile_pool(name="rsbp", bufs=2) as rsbp, \
             tc.tile_pool(name="qop", bufs=2) as qop, \
             tc.tile_pool(name="otp", bufs=1) as otp, \
             tc.tile_pool(name="outp", bufs=2) as outp:

            def attn_col(j, h0=0, h1=NH):
                jc = slice(j * 512, (j + 1) * 512)
                nk = 4 * (j + 1)
                for h in range(h0, h1):
                    kv = h // (NH // NKV)
                    a_t, pr = h // 2, (h % 2) * 64
                    qh = qT[a_t][pr:pr + 64, :]
                    ksrc = kT if kv * HD == pr else kT2
                    kh = ksrc[pr:pr + 64, :]
                    ps_o = pso.tile([128, 512], F32, tag="o")
                    ss, pts = {}, {}

                    def emit_s(r):
                        phi = r - 4 * j
                        c0 = 128 * phi if phi > 0 else 0
                        t = psa.tile([128, 512], F32, tag="s")
                        nc.tensor.matmul(
                            t[:, c0:512], kh[:, r * 128:(r + 1) * 128],
                            qh[:, j * 512 + c0:(j + 1) * 512],
                            start=True, stop=True)
                        ss[r] = (t, c0)

                    def emit_exp(r):
                        t, c0 = ss.pop(r)
                        pt = ptp.tile([128, 512], BF16, tag="pt")
                        nc.scalar.activation(pt[:, c0:512], t[:, c0:512],
                                             AF.Exp, bias=zero_col, scale=1.0)
                        if r - 4 * j >= 0:
                            nc.vector.tensor_tensor(
                                out=pt[:, c0:c0 + 128],
                                in0=pt[:, c0:c0 + 128],
                                in1=trimask, op=ALU.mult)
                        pts[r] = (pt, c0)

                    def emit_v(r):
                        pt, c0 = pts.pop(r)
                        # columns < c0 are fully masked: skip them instead of
                        # zero-filling (they were started by earlier k-tiles)
                        nc.tensor.matmul(ps_o[0:HD + 1, c0:512],
                                         v1[kv][r], pt[:, c0:512],
                                         start=(r == 0), stop=(r == nk - 1),
                                         skip_group_check=True)

                    emit_s(0)
                    emit_exp(0)
                    if nk > 1:
                        emit_s(1)
                        emit_exp(1)
                    for r in range(2, nk):
                        emit_s(r)
                        emit_exp(r)
                        emit_v(r - 2)
                    for r in (nk - 2, nk - 1):
                        if r >= 0 and r in pts:
                            emit_v(r)

                    sumstage = sump.tile([1, 512], F32, tag="sumstage")
                    nc.scalar.copy(out=sumstage, in_=ps_o[HD:HD + 1, :])
                    nc.sync.dma_start(out=sums_d[h:h + 1, jc], in_=sumstage)
                    nc.vector.tensor_scalar_mul(ao[a_t][pr:pr + 64, jc],
                                                ps_o[0:HD, :],
                                                go_cols[:, h:h + 1])

            def ao_stats_pre(c):
                # normalize + local stats + stats AllGather launch; no op here
                # waits on a collective, so it can sit anywhere in the queues
                js, je = chunk_js[c]
                W = cw[c]
                CT = W // 128
                th = slice(js * 512, je * 512)
                ih = slice(js * 4, je * 4)
                sums_sb = aosc.tile([NH, W_MAX], F32, tag="sums", name="sums_sb")[:, 0:W]
                nc.sync.dma_start(out=sums_sb, in_=sums_d[:, th])
                nc.vector.reciprocal(sums_sb, sums_sb)
                nc.sync.dma_start(out=rsums_d[:, th], in_=sums_sb)
                # normalize ao in place (transposed layout, per-column rsums)
                for a in range(NO):
                    rsb = rsbp.tile([128, W_MAX], BF16, tag="rsb", name="rsb")[:, 0:W]
                    nc.gpsimd.dma_start(
                        out=rsb[0:64, :],
                        in_=rsums_d[2 * a:2 * a + 1, th].to_broadcast(
                            (64, W)))
                    nc.gpsimd.dma_start(
                        out=rsb[64:128, :],
                        in_=rsums_d[2 * a + 1:2 * a + 2, th].to_broadcast(
                            (64, W)))
                    nc.vector.tensor_tensor(out=ao[a][:, th],
                                            in0=ao[a][:, th],
                                            in1=rsb, op=ALU.mult)
                # per-token stats over the local 512 dims via PE transpose
                for i in range(js * 4, je * 4):
                    ci = slice(i, i + 1)
                    pst = pst_pool.tile([128, 512], BF16, tag="pst")
                    for a in range(NO):
                        nc.tensor.transpose(pst[:, a * 128:(a + 1) * 128],
                                            ao[a][:, i * 128:(i + 1) * 128],
                                            identity)
                    sq_scr = aosc.tile([128, 512], BF16, tag="aosq")
                    nc.scalar.activation(sq_scr, pst, AF.Square,
                                         bias=zero_col, scale=1.0,
                                         accum_out=ss_o_col[:, ci])
                    nc.vector.tensor_reduce(out=amax_o_col[:, ci], in_=pst,
                                            axis=mybir.AxisListType.X,
                                            op=ALU.max,
                                            apply_absolute_value=True)
                # pack partial stats, AllGather to [512, 2*CT]
                stats_sb = aosc.tile([128, 2 * CT_MAX], F32, tag="spack",
                                     name="spack")[:, 0:2 * CT]
                nc.vector.tensor_copy(out=stats_sb[:, 0:CT],
                                      in_=ss_o_col[:, ih])
                nc.vector.tensor_copy(out=stats_sb[:, CT:2 * CT],
                                      in_=amax_o_col[:, ih])
                nc.sync.dma_start(out=stats_in[c][:], in_=stats_sb)
                if emulate_collectives:
                    for p in range(G):
                        nc.sync.dma_start(
                            out=stats_out[c][p * 128:(p + 1) * 128, :],
                            in_=stats_in[c][:])
                else:
                    nc.gpsimd.collective_compute("AllGather", ALU.bypass,
                                                 replica_groups=rg,
                                                 ins=[stats_in[c].opt()],
                                                 outs=[stats_out[c].opt()])

            def ao_stats_post(c):
                # combine gathered stats + quant-scale math; the head of this
                # chain waits on the stats AllGather, so it is emitted half an
                # attention column after ao_stats_pre(c)
                js, je = chunk_js[c]
                W = cw[c]
                CT = W // 128
                th = slice(js * 512, je * 512)
                ih = slice(js * 4, je * 4)
                parts = [aosc.tile([128, 2 * CT_MAX], F32, tag=f"parts{p}",
                                   name=f"parts{p}")[:, 0:2 * CT]
                         for p in range(G)]
                for p in range(G):
                    nc.sync.dma_start(
                        out=parts[p],
                        in_=stats_out[c][p * 128:(p + 1) * 128, :])
                for p in range(1, G):
                    nc.vector.tensor_tensor(out=parts[0][:, 0:CT],
                                            in0=parts[0][:, 0:CT],
                                            in1=parts[p][:, 0:CT],
                                            op=ALU.add)
                    nc.vector.tensor_tensor(out=parts[0][:, CT:2 * CT],
                                            in0=parts[0][:, CT:2 * CT],
                                            in1=parts[p][:, CT:2 * CT],
                                            op=ALU.max)
                ss_full = parts[0][:, 0:CT]
                amax_full = parts[0][:, CT:2 * CT]
                rsq_o = aosc.tile([128, CT_MAX], F32, tag="rsqo", name="rsq_o")[:, 0:CT]
                xsc_o = aosc.tile([128, CT_MAX], F32, tag="xsco", name="xsc_o")[:, 0:CT]
                scr2 = aosc.tile([128, CT_MAX], F32, tag="scr2", name="scr2")[:, 0:CT]
                nc.scalar.activation(scr2, ss_full, AF.Sqrt, bias=epsn_col,
                                     scale=1.0 / (H_TOTAL * HD))
                nc.vector.reciprocal(rsq_o, scr2)
                nc.vector.tensor_tensor(out=xsc_o, in0=amax_full, in1=rsq_o,
                                        op=ALU.mult)
                nc.vector.tensor_scalar_max(xsc_o, xsc_o, EPS_Q)
                nc.vector.reciprocal(scr2, xsc_o)
                nc.vector.tensor_tensor(out=so_col[:, ih], in0=rsq_o,
                                        in1=scr2, op=ALU.mult)
                nc.vector.tensor_scalar_mul(so_col[:, ih], so_col[:, ih],
                                            127.0)
                nc.vector.tensor_scalar(fo_col[:, ih], xsc_o,
                                        ws_cols[:, 3:4], 1.0 / 127.0,
                                        op0=ALU.mult, op1=ALU.mult)
                nc.sync.dma_start(
                    out=so_d[0:1, th].rearrange("1 (i p) -> p i", p=128),
                    in_=so_col[:, ih])

            def ao_quant_gather(c):
                js, je = chunk_js[c]
                W = cw[c]
                th = slice(js * 512, je * 512)
                sob = rsbp.tile([128, W_MAX], BF16, tag="sob", name="sob")[:, 0:W]
                nc.gpsimd.dma_start(out=sob,
                                    in_=so_d[0:1, th].to_broadcast((128, W)))
                for a in range(NO):
                    tmp = qop.tile([128, W_MAX], F32, tag="qtmp", name="qtmp")[:, 0:W]
                    nc.vector.tensor_tensor(out=tmp, in0=ao[a][:, th],
                                            in1=sob, op=ALU.mult)
                    xqo = qop.tile([128, W_MAX], BF16, tag="xqo", name="xqo")[:, 0:W]
                    nc.vector.tensor_scalar(xqo, tmp, MAGIC, MAGIC,
                                            op0=ALU.add, op1=ALU.subtract)
                    xqo8 = qop.tile([128, W_MAX], I8, tag="xqo8", name="xqo8")[:, 0:W]
                    nc.vector.tensor_copy(out=xqo8, in_=xqo)
                    nc.sync.dma_start(
                        out=xqo_in[c][a * 128:(a + 1) * 128, :], in_=xqo8)
                if emulate_collectives:
                    for p in range(G):
                        nc.sync.dma_start(
                            out=xqo_out[c][p * QO:(p + 1) * QO, :],
                            in_=xqo_in[c][:])
                else:
                    nc.gpsimd.collective_compute("AllGather", ALU.bypass,
                                                 replica_groups=rg,
                                                 ins=[xqo_in[c].opt()],
                                                 outs=[xqo_out[c].opt()])

            def oproj(c):
                js, je = chunk_js[c]
                ot = [otp.tile([128, W_MAX], BF16, tag=f"ot{r}",
                               name=f"ot{r}")[:, 0:cw[c]]
                      for r in range(ND)]
                for r in range(ND):
                    # SWDGE cast-DMA int8 -> bf16, bypasses HWDGE
                    nc.gpsimd.dma_start(
                        out=ot[r], in_=xqo_out[c][r * 128:(r + 1) * 128, :])
                for il in range(cw[c] // 128):
                    i = js * 4 + il
                    ps_f = psf.tile([128, 512], F32, tag="f")
                    for r in range(ND):
                        nc.tensor.matmul(ps_f,
                                         ot[r][:, il * 128:(il + 1) * 128],
                                         wt[r][:, C_O:C_O + QO],
                                         start=(r == 0), stop=(r == ND - 1))
                    out_t = outp.tile([128, QO], F32, tag="out")
                    nc.scalar.activation(out_t, ps_f, AF.Copy, bias=0.0,
                                         scale=fo_col[:, i:i + 1])
                    nc.sync.dma_start(out=out_d[i * 128:(i + 1) * 128, :],
                                      in_=out_t)

            for j in range(chunk_js[0][0], chunk_js[0][1]):
                attn_col(j)
            for c in range(NCK):
                ao_stats_pre(c)
                if c + 1 < NCK:
                    js, je = chunk_js[c + 1]
                    # split the next chunk's first column by heads so the
                    # in-order queues reach post/quant only after the stats
                    # AllGather has had half a column of attention to finish
                    attn_col(js, 0, NH // 2)
                    ao_stats_post(c)
                    ao_quant_gather(c)
                    attn_col(js, NH // 2, NH)
                    for j in range(js + 1, je):
                        attn_col(j)
                else:
                    # scheduling-time floors keep the gather-gated
                    # o-projections out of the attention engine queues
                    for cc in range(NCK - 1):
                        with tc.tile_wait_until(1.0 + 0.25 * cc):
                            oproj(cc)
                    ao_stats_post(c)
                    ao_quant_gather(c)
                    with tc.tile_wait_until(1.0 + 0.25 * (NCK - 1)):
                        oproj(c)

        es_qkv.close()  # frees qT, kT, kT2, v1
        es_ao.close()   # frees ao
        es_w.close()

    nc.compile()
    return nc


# ---------------------------------------------------------------------------
# host wrapper
# ---------------------------------------------------------------------------
_CACHE = {}


def _get_program(T, has_g):
    key = (T, has_g)
    if key not in _CACHE:
        _CACHE[key] = build_program(T=T, has_g=has_g)
    return _CACHE[key]


def _tern(w):
    ws = max(np.mean(np.abs(w), dtype=np.float32), np.float32(EPS_Q))
    wq = np.clip(np.rint(w / ws), -1.0, 1.0).astype(np.float32)
    return wq, float(ws)


def make_in_maps(x, cos, sin, wq, wk, wv, wo, gq, gk, gv, go, T):
    import ml_dtypes
    BF = ml_dtypes.bfloat16
    cosT = np.ascontiguousarray(cos.T.astype(np.float32))      # [64, T]
    sinT = np.ascontiguousarray(sin.T.astype(np.float32))
    cost2 = np.concatenate([cosT, cosT], axis=0)               # [128, T]
    sint_signed = np.concatenate([-sinT[0:32], sinT[32:64]], axis=0)
    sint2s = np.concatenate([sint_signed, sint_signed], axis=0)
    cost2 = cost2.astype(BF)
    sint2s = sint2s.astype(BF)

    # rotate-half permutation (pure swap; sign lives in sint2s)
    pswap = np.zeros((128, 128), np.float32)
    for p in range(128):
        blk, off = (p // 64) * 64, p % 64
        pswap[p, blk + (off + 32) % 64] = 1.0
    pswap = pswap.astype(BF)

    wq_t, wsq = _tern(np.asarray(wq, np.float32))
    wk_t, wsk = _tern(np.asarray(wk, np.float32))
    wv_t, wsv = _tern(np.asarray(wv, np.float32))
    wo_t, wso = _tern(np.asarray(wo, np.float32))
    ws_r = np.array([[wsq, wsk, wsv, wso]], np.float32)

    ones = np.ones((D,), np.float32)
    has_g = not (np.array_equal(gq, ones) and np.array_equal(gk, ones)
                 and np.array_equal(gv, ones))
    if has_g:
        assert np.array_equal(gq, gk) and np.array_equal(gk, gv), \
            "per-projection norm weights must match"

    in_maps = []
    for c in range(N_CORES):
        b, g = c // G, c % G
        wcat = np.concatenate([
            wq_t[g * QO:(g + 1) * QO, :].T,
            wk_t[g * KO:(g + 1) * KO, :].T,
            wv_t[g * KO:(g + 1) * KO, :].T,
            wo_t[g * QO:(g + 1) * QO, :].T,
        ], axis=1).astype(BF)
        m = {
            "x": np.ascontiguousarray(x[b].astype(np.float32)),
            "wcat": np.ascontiguousarray(wcat),
            "cost2": cost2,
            "sint2s": sint2s,
            "pswap": pswap,
            "ws_r": ws_r,
            "go_r": np.ascontiguousarray(go[g * QO:(g + 1) * QO][None, :]),
        }
        if has_g:
            m["g_r"] = np.ascontiguousarray(gq[None, :])
        in_maps.append(m)
    return in_maps, has_g


def assemble_output(results, T):
    out = np.empty((2, T, D), np.float32)
    for c in range(N_CORES):
        b, g = c // G, c % G
        out[b][:, g * QO:(g + 1) * QO] = results[c]["out"]
    return out


def kernel(x, cos, sin, wq, wk, wv, wo, gq, gk, gv, go):
    x = np.asarray(x, np.float32)
    T = x.shape[1]
    in_maps, has_g = make_in_maps(x, cos, sin, np.asarray(wq, np.float32),
                                  np.asarray(wk, np.float32),
                                  np.asarray(wv, np.float32),
                                  np.asarray(wo, np.float32),
                                  np.asarray(gq, np.float32),
                                  np.asarray(gk, np.float32),
                                  np.asarray(gv, np.float32),
                                  np.asarray(go, np.float32), T)
    nc = _get_program(T, has_g)
    res = bass_utils.run_bass_kernel_spmd(nc, in_maps,
                                          core_ids=list(range(N_CORES)))
    return assemble_output(res.results, T)



# revision 22
# speedup vs baseline: 83.7834x; 1.0865x over previous
"""BitGQA attention kernel for 8 trn2 NeuronCores.

Sharding: 8 cores = 2 batch groups x 4 tensor-parallel groups.
Core c handles batch b = c//4 and head-group g = c%4 (8 q heads, 2 kv heads,
512-wide slices of the q/o projections, 128-wide slices of k/v).

Host prep (once per weight set, outside the timed device program):
  - ternary-quantize all four projection weights (exact reference math:
    clip(round(w/mean|w|), -1, 1)) and ship them as one concatenated
    transposed bf16 matrix [2048, 512+128+128+512] per rank
  - ship the 4 w_scales, rope tables in transposed halved layout, and the
    rotate-half permutation matrix

Device dataflow (per core), activations transposed [feature, token] so the
contraction dim sits on partitions:
  1. stream x token tiles: row stats (ACT square-accum + Pool abs-max),
     absmax-quantize via fp32 magic rounding, ONE strided 3D xbar-transpose
     per tile into the column-interleaved xqT layout
  2. q/k/v projections (bf16 matmuls, exact integer x ternary); v computed
     in transposed form (one wide matmul per d-tile) then PE-transposed
     into the [token, hd] attention layout with the per-token dequant
     scale applied on DVE; RoPE applied as q*cosf + (P@q)*sinf where P is
     a permutation matmul and the cos/sin tables have the per-token
     dequant scales folded in
  3. causal attention per head, software-pipelined: rotating PSUM score
     banks so PE score-matmuls run ahead of ACT exp; softmax denominator
     falls out of the same matmul via the [v|1] augmented operand
  4. attention-output normalization in transposed layout, stats via PE
     transpose into one PSUM bank (ACT square-accum + DVE absmax straight
     from PSUM), tiny stats AllGather, absmax-quantize to int8
  5. int8 AllGather of the o-projection input in 2 token-halves overlapped
     with the o-projection matmuls (SWDGE cast-DMAs int8->bf16)
  6. o-projection, per-token rescale, write the [t, 512] slice.

The final output is assembled on host from the 8 [2048, 512] slices.
"""

import contextlib

import numpy as np

import concourse.bass as bass
import concourse.bacc as bacc
import concourse.mybir as mybir
import concourse.tile as tile
from concourse import bass_utils

F32 = mybir.dt.float32
BF16 = mybir.dt.bfloat16
I8 = mybir.dt.int8
AF = mybir.ActivationFunctionType
ALU = mybir.AluOpType

MAGIC = float(1.5 * 2.0**23)  # fp32 round-to-nearest-even magic constant
EPS_NORM = 1e-6
EPS_Q = 1e-5

N_CORES = 8
D = 2048
H_TOTAL, KV_TOTAL, HD = 32, 8, 64
G = 4  # tensor-parallel groups
NH = H_TOTAL // G          # 8 local q heads
NKV = KV_TOTAL // G        # 2 local kv heads
QO = NH * HD               # 512 local q dims
KO = NKV * HD              # 128 local kv dims
ND = D // 128              # 16 d-tiles
WCAT = QO + KO + KO + QO   # 1280 concatenated weight columns
C_Q, C_K, C_V, C_O = 0, QO, QO + KO, QO + 2 * KO


def build_program(T=2048, has_g=False, n_cores=N_CORES,
                  emulate_collectives=False, n_chunks=2, vt_v=True,
                  driver="orig", exp_pair=False, floor0=1.0):
    NT = T // 128   # token tiles
    NJ = T // 512   # 512-wide token columns
    NO = QO // 128  # q/o o-tiles (4)
    TH = T // 2     # token half for the chunked o-gather
    NTH = NT // 2
    rg = ([[0, 1, 2, 3], [4, 5, 6, 7]] if n_cores == N_CORES else
          [[c] for c in range(n_cores)])

    nc = bacc.Bacc("TRN2", target_bir_lowering=False, debug=False,
                   num_devices=n_cores)

    # ---- per-core DRAM I/O ----
    x_d = nc.dram_tensor("x", [T, D], F32, kind="ExternalInput")
    wcat_d = nc.dram_tensor("wcat", [D, WCAT], BF16, kind="ExternalInput")
    cost2_d = nc.dram_tensor("cost2", [128, T], BF16, kind="ExternalInput")
    sint2s_d = nc.dram_tensor("sint2s", [128, T], BF16, kind="ExternalInput")
    pswap_d = nc.dram_tensor("pswap", [128, 128], BF16, kind="ExternalInput")
    ws_d = nc.dram_tensor("ws_r", [1, 4], F32, kind="ExternalInput")
    go_d = nc.dram_tensor("go_r", [1, QO], F32, kind="ExternalInput")
    if has_g:
        g_d = nc.dram_tensor("g_r", [1, D], F32, kind="ExternalInput")
    out_d = nc.dram_tensor("out", [T, QO], F32, kind="ExternalOutput")

    with tile.TileContext(nc) as tc, contextlib.ExitStack() as stack:
        # ---------------- long-lived pools (strict stack order) --------------
        singles = stack.enter_context(tc.tile_pool(name="singles", bufs=1))
        cols = stack.enter_context(tc.tile_pool(name="cols", bufs=1))
        dram = stack.enter_context(tc.tile_pool(name="dram", bufs=1,
                                                space="DRAM"))

        # constants
        zero_col = singles.tile([128, 1], F32)
        nc.vector.memset(zero_col, 0.0)
        magic_col = singles.tile([128, 1], F32)
        nc.vector.memset(magic_col, MAGIC)
        epsn_col = singles.tile([128, 1], F32)
        nc.vector.memset(epsn_col, EPS_NORM)

        identity = singles.tile([128, 128], BF16)
        nc.gpsimd.memset(identity, 1.0)
        nc.gpsimd.affine_select(out=identity, in_=identity, compare_op=ALU.is_ge,
                                fill=0.0, base=0, pattern=[[-1, 128]],
                                channel_multiplier=1)
        nc.gpsimd.affine_select(out=identity, in_=identity, compare_op=ALU.is_ge,
                                fill=0.0, base=0, pattern=[[1, 128]],
                                channel_multiplier=-1)

        # causal mask for diagonal 128x128 blocks of scoresT[k, t]:
        # keep 1.0 where t >= k i.e. (free - partition) >= 0
        trimask = singles.tile([128, 128], BF16)
        nc.gpsimd.memset(trimask, 1.0)
        nc.gpsimd.affine_select(out=trimask, in_=trimask, compare_op=ALU.is_ge,
                                fill=0.0, base=0, pattern=[[1, 128]],
                                channel_multiplier=-1)

        pswap = singles.tile([128, 128], BF16)
        nc.sync.dma_start(out=pswap, in_=pswap_d[:, :])

        # go as per-head columns [64, NH]
        go_cols = singles.tile([64, NH], F32)
        nc.sync.dma_start(out=go_cols,
                          in_=go_d[0:1, :].rearrange("1 (h p) -> p h", p=64))

        ws_cols = singles.tile([128, 4], F32)
        nc.sync.dma_start(out=ws_cols, in_=ws_d[0:1, :].to_broadcast((128, 4)))

        if has_g:
            g_bcast = singles.tile([128, D], F32)
            nc.sync.dma_start(out=g_bcast,
                              in_=g_d[0:1, :].to_broadcast((128, D)))

        # ====== lifetime pools, opened in reverse-close order ================
        es_w = contextlib.ExitStack()
        w_pool = es_w.enter_context(tc.tile_pool(name="wp", bufs=1))
        es_ao = contextlib.ExitStack()
        ao_pool = es_ao.enter_context(tc.tile_pool(name="aop", bufs=1))
        es_qkv = contextlib.ExitStack()
        qkv_pool = es_qkv.enter_context(tc.tile_pool(name="qkv", bufs=1))
        es_proj = contextlib.ExitStack()
        proj_pool = es_proj.enter_context(tc.tile_pool(name="proj", bufs=1))

        # quantized weights: 16 tiles [128, 1280] (q|k|v|o column blocks)
        wt = [w_pool.tile([128, WCAT], BF16, name=f"wt{r}") for r in range(ND)]
        for r in range(ND):
            nc.sync.dma_start(out=wt[r], in_=wcat_d[r * 128:(r + 1) * 128, :])

        cost2 = proj_pool.tile([128, T], BF16)
        nc.sync.dma_start(out=cost2, in_=cost2_d[:, :])
        sint2s = proj_pool.tile([128, T], BF16)
        nc.sync.dma_start(out=sint2s, in_=sint2s_d[:, :])

        # attention operands (qkv lifetime)
        qT = [qkv_pool.tile([128, T], BF16, name=f"qT{a}") for a in range(NO)]
        kT = qkv_pool.tile([128, T], BF16)
        v1 = [[qkv_pool.tile([128, HD + 1], BF16, name=f"v1_{kv}_{r}")
               for r in range(NT)] for kv in range(NKV)]
        for kv in range(NKV):
            for r in range(NT):
                nc.vector.memset(v1[kv][r][:, HD:HD + 1], 1.0)

        # x-quant stat columns
        ss_col = cols.tile([128, NT], F32)
        amax_col = cols.tile([128, NT], F32)
        rsq_col = cols.tile([128, NT], F32)
        xsc_col = cols.tile([128, NT], F32)
        s_col = cols.tile([128, NT], F32)
        scr1_col = cols.tile([128, NT], F32)
        fv_col = cols.tile([128, NT], F32)
        xsc_d = dram.tile([1, T], F32)

        # -------- phases X+P merged: stream token columns of 512 -------------
        with tc.tile_pool(name="xpool", bufs=1 if has_g else 2) as xpool, \
             tc.tile_pool(name="xscr", bufs=1) as xscr, \
             tc.tile_pool(name="xqTc", bufs=1 if has_g else 2) as xqTc_pool, \
             tc.tile_pool(name="fqfp", bufs=1) as fqfp, \
             tc.tile_pool(name="rawp", bufs=1 if has_g else 2) as rawp, \
             tc.tile_pool(name="psq", bufs=1, space="PSUM") as psq, \
             tc.tile_pool(name="psk", bufs=1, space="PSUM") as psk, \
             tc.tile_pool(name="psv", bufs=1, space="PSUM") as psv, \
             tc.tile_pool(name="psro", bufs=2, space="PSUM") as psro:
            for j in range(NJ):
                jc = slice(j * 512, (j + 1) * 512)
                # interleaved layout: block r at cols [r*512+s*128, +128)
                xqTc = xqTc_pool.tile([128, ND * 512], BF16, tag="xqTc")
                xqTc_v = xqTc.rearrange("p (r s c) -> p r s c", r=ND, s=4)
                # ---- X: quantize 4 token tiles of this column ----
                cj = slice(4 * j, 4 * j + 4)
                xts = []
                for s4 in range(4):
                    i = 4 * j + s4
                    ci = slice(i, i + 1)
                    xt = xpool.tile([128, D], F32, tag=f"xt{s4}",
                                    name=f"xt{s4}", bufs=1)
                    nc.sync.dma_start(out=xt,
                                      in_=x_d[i * 128:(i + 1) * 128, :])
                    if has_g:
                        xg = xpool.tile([128, D], F32, tag=f"xg{s4}",
                                        name=f"xg{s4}", bufs=1)
                        nc.vector.tensor_tensor(out=xg, in0=xt, in1=g_bcast,
                                                op=ALU.mult)
                        src = xg
                    else:
                        src = xt
                    xts.append(src)
                    # discard target for Square reuses the scratch slot bytes
                    sq_scr = xscr.tile([128, D], F32, tag="xtmp")
                    nc.scalar.activation(sq_scr.bitcast(BF16)[:, 0:D], xt,
                                         AF.Square, bias=zero_col,
                                         scale=1.0, accum_out=ss_col[:, ci])
                    nc.vector.tensor_reduce(out=amax_col[:, ci], in_=src,
                                            axis=mybir.AxisListType.X,
                                            op=ALU.max,
                                            apply_absolute_value=True)
                # batched per-column stat math on [128, 4] slices
                nc.scalar.activation(scr1_col[:, cj], ss_col[:, cj],
                                     AF.Sqrt, bias=epsn_col, scale=1.0 / D)
                nc.vector.reciprocal(rsq_col[:, cj], scr1_col[:, cj])
                nc.vector.tensor_tensor(out=xsc_col[:, cj],
                                        in0=amax_col[:, cj],
                                        in1=rsq_col[:, cj], op=ALU.mult)
                nc.vector.tensor_scalar_max(xsc_col[:, cj], xsc_col[:, cj],
                                            EPS_Q)
                nc.vector.reciprocal(scr1_col[:, cj], xsc_col[:, cj])
                nc.vector.tensor_tensor(out=s_col[:, cj], in0=rsq_col[:, cj],
                                        in1=scr1_col[:, cj], op=ALU.mult)
                nc.vector.tensor_scalar_mul(s_col[:, cj], s_col[:, cj], 127.0)
                nc.vector.tensor_scalar(fv_col[:, cj], xsc_col[:, cj],
                                        ws_cols[:, 2:3], 1.0 / 127.0,
                                        op0=ALU.mult, op1=ALU.mult)
                for s4 in range(4):
                    i = 4 * j + s4
                    tmp = xscr.tile([128, D], F32, tag="xtmp")
                    nc.scalar.activation(tmp, xts[s4], AF.Identity,
                                         bias=magic_col,
                                         scale=s_col[:, i:i + 1])
                    xq = xpool.tile([128, D], BF16, tag="xq")
                    nc.vector.tensor_scalar_sub(xq, tmp, MAGIC)
                    # one xbar transpose for all 16 d-tiles of this token tile
                    nc.sync.dma_start_transpose(
                        out=xqTc_v[:, :, s4, :], in_=xq)

                # ---- per-token dequant factors folded into rope tables ----
                nc.sync.dma_start(
                    out=xsc_d[0:1, jc].rearrange("1 (i p) -> p i", p=128),
                    in_=xsc_col[:, cj])
                fq_f = fqfp.tile([128, 512], F32, tag="fqf")
                nc.sync.dma_start(out=fq_f,
                                  in_=xsc_d[0:1, jc].to_broadcast((128, 512)))
                xf_q = fqfp.tile([128, 512], BF16, tag="xfq")
                xf_k = fqfp.tile([128, 512], BF16, tag="xfk")
                nc.vector.tensor_scalar(xf_q, fq_f, ws_cols[:, 0:1],
                                        1.0 / 127.0, op0=ALU.mult, op1=ALU.mult)
                nc.vector.tensor_scalar(xf_k, fq_f, ws_cols[:, 1:2],
                                        1.0 / (127.0 * float(np.sqrt(HD))),
                                        op0=ALU.mult, op1=ALU.mult)
                cosq = fqfp.tile([128, 512], BF16, tag="cosq")
                sinq = fqfp.tile([128, 512], BF16, tag="sinq")
                cosk = fqfp.tile([128, 512], BF16, tag="cosk")
                sink = fqfp.tile([128, 512], BF16, tag="sink")
                nc.vector.tensor_tensor(out=cosq, in0=cost2[:, jc], in1=xf_q,
                                        op=ALU.mult)
                nc.vector.tensor_tensor(out=sinq, in0=sint2s[:, jc], in1=xf_q,
                                        op=ALU.mult)
                nc.vector.tensor_tensor(out=cosk, in0=cost2[:, jc], in1=xf_k,
                                        op=ALU.mult)
                nc.vector.tensor_tensor(out=sink, in0=sint2s[:, jc], in1=xf_k,
                                        op=ALU.mult)

                # ---- P: projections for this column ----
                ps_q = [psq.tile([128, 512], F32, tag=f"q{a}", name=f"ps_q{a}")
                        for a in range(NO)]
                ps_k = psk.tile([128, 512], F32)
                ps_v = psv.tile([128, 512], F32)
                for r in range(ND):
                    ch = xqTc[:, r * 512:(r + 1) * 512]
                    st = dict(start=(r == 0), stop=(r == ND - 1))
                    for a in range(NO):
                        nc.tensor.matmul(
                            ps_q[a],
                            wt[r][:, C_Q + a * 128:C_Q + (a + 1) * 128],
                            ch, **st)
                    nc.tensor.matmul(ps_k, wt[r][:, C_K:C_K + KO], ch, **st)
                    if vt_v:
                        # v in transposed form: one wide matmul per d-tile
                        nc.tensor.matmul(ps_v, wt[r][:, C_V:C_V + KO], ch,
                                         **st)
                if vt_v:
                    # vT -> v1 via PE transpose; fv (per-token) on DVE
                    vT_sb = rawp.tile([128, 512], BF16, tag="vtsb")
                    nc.scalar.copy(out=vT_sb, in_=ps_v)
                    ps_t = psro.tile([128, 512], BF16, tag="sh")
                    for s in range(4):
                        nc.tensor.transpose(ps_t[:, s * 128:(s + 1) * 128],
                                            vT_sb[:, s * 128:(s + 1) * 128],
                                            identity)
                    for s in range(4):
                        kt_i = 4 * j + s
                        for kv in range(NKV):
                            nc.vector.tensor_scalar_mul(
                                v1[kv][kt_i][:, 0:HD],
                                ps_t[:, s * 128 + kv * HD:
                                     s * 128 + (kv + 1) * HD],
                                fv_col[:, kt_i:kt_i + 1])
                else:
                    for s in range(4):
                        for r in range(ND):
                            nc.tensor.matmul(
                                ps_v[:, s * 128:(s + 1) * 128],
                                xqTc[:, r * 512 + s * 128:
                                     r * 512 + (s + 1) * 128],
                                wt[r][:, C_V:C_V + KO], start=(r == 0),
                                stop=(r == ND - 1))
                    for s in range(4):
                        kt_i = 4 * j + s
                        for kv in range(NKV):
                            nc.scalar.activation(
                                v1[kv][kt_i][:, 0:HD],
                                ps_v[:, s * 128 + kv * HD:
                                     s * 128 + (kv + 1) * HD],
                                AF.Copy, bias=0.0,
                                scale=fv_col[:, kt_i:kt_i + 1])

                # ---- rope: dst = raw*cosf + (P@raw)*sinf ----
                def rope(dst, ps_raw, cosf, sinf):
                    raw = rawp.tile([128, 512], BF16, tag="raw")
                    nc.scalar.copy(out=raw, in_=ps_raw)
                    ps_sh = psro.tile([128, 512], F32, tag="sh")
                    nc.tensor.matmul(ps_sh, pswap, raw, start=True, stop=True)
                    sh = rawp.tile([128, 512], BF16, tag="sh")
                    nc.vector.tensor_tensor(out=sh, in0=ps_sh, in1=sinf,
                                            op=ALU.mult)
                    cq = rawp.tile([128, 512], BF16, tag="cq")
                    nc.vector.tensor_tensor(out=cq, in0=raw, in1=cosf,
                                            op=ALU.mult)
                    nc.vector.tensor_tensor(out=dst, in0=cq, in1=sh,
                                            op=ALU.add)

                for a in range(NO):
                    rope(qT[a][:, jc], ps_q[a], cosq, sinq)
                rope(kT[:, jc], ps_k, cosk, sink)

        es_proj.close()  # frees rope tables, xqT column tiles

        # kT with kv halves swapped so every q head finds its kv head at its
        # own base partition (matmul requires equal base partitions)
        kT2 = qkv_pool.tile([128, T], BF16)
        nc.vector.tensor_copy(out=kT2[0:64, :], in_=kT[64:128, :])
        nc.vector.tensor_copy(out=kT2[64:128, :], in_=kT[0:64, :])

        ao = [ao_pool.tile([128, T], BF16, name=f"ao{a}") for a in range(NO)]
        sums_d = dram.tile([NH, T], F32)
        rsums_d = dram.tile([NH, T], F32)

        # token chunks for the AO/gather/o-proj pipeline: chunk c's
        # stats-AllGather + quantize + int8-gather hide under chunk c+1's
        # attention, and its o-projection under chunk c+2's
        if driver == "orig":
            chunk_js = ([(0, NJ // 2), (NJ // 2, NJ)] if NJ >= 2
                        else [(0, NJ)])
        else:
            ncicol = max(1, NJ // n_chunks)
            chunk_js = [(j, min(j + ncicol, NJ))
                        for j in range(0, NJ, ncicol)]
        NCK = len(chunk_js)
        cw = [(je - js) * 512 for js, je in chunk_js]
        W_MAX = max(cw)
        CT_MAX = W_MAX // 128

        # AO stat columns (full-T width, filled per token chunk)
        ss_o_col = cols.tile([128, NT], F32)
        amax_o_col = cols.tile([128, NT], F32)
        so_col = cols.tile([128, NT], F32)
        fo_col = cols.tile([128, NT], F32)
        so_d = dram.tile([1, T], F32)
        xqo_in = [dram.tile([QO, cw[c]], I8, name=f"xqoin{c}")
                  for c in range(NCK)]
        xqo_out = [dram.tile([G * QO, cw[c]], I8, name=f"xqoout{c}")
                   for c in range(NCK)]
        stats_in = [dram.tile([128, cw[c] // 64], F32, name=f"stin{c}")
                    for c in range(NCK)]
        stats_out = [dram.tile([128 * G, cw[c] // 64], F32, name=f"stout{c}")
                     for c in range(NCK)]

        # ------- phases A/AO/O interleaved: attention on token-half 1 -------
        # overlaps stats + quantization + int8 AllGather of token-half 0
        with tc.tile_pool(name="psa", bufs=2 if exp_pair else 3,
                          space="PSUM") as psa, \
             tc.tile_pool(name="pso", bufs=2, space="PSUM") as pso, \
             tc.tile_pool(name="pst", bufs=1, space="PSUM") as pst_pool, \
             tc.tile_pool(name="psf", bufs=1 if exp_pair else 2,
                          space="PSUM") as psf, \
             tc.tile_pool(name="ptp", bufs=6) as ptp, \
             tc.tile_pool(name="sump", bufs=16) as sump, \
             tc.tile_pool(name="aosc", bufs=2) as aosc, \
             tc.tile_pool(name="rsbp", bufs=2) as rsbp, \
             tc.tile_pool(name="qop", bufs=2) as qop, \
             tc.tile_pool(name="otp", bufs=1) as otp, \
             tc.tile_pool(name="outp", bufs=2) as outp:

            def attn_head_tail(h, j, ps_o):
                jc = slice(j * 512, (j + 1) * 512)
                a_t, pr = h // 2, (h % 2) * 64
                sumstage = sump.tile([1, 512], F32, tag="sumstage")
                nc.scalar.copy(out=sumstage, in_=ps_o[HD:HD + 1, :])
                nc.sync.dma_start(out=sums_d[h:h + 1, jc], in_=sumstage)
                nc.vector.tensor_scalar_mul(ao[a_t][pr:pr + 64, jc],
                                            ps_o[0:HD, :],
                                            go_cols[:, h:h + 1])

            def attn_col_single(j, h0, h1):
                nk = 4 * (j + 1)
                for h in range(h0, h1):
                    kv = h // (NH // NKV)
                    a_t, pr = h // 2, (h % 2) * 64
                    qh = qT[a_t][pr:pr + 64, :]
                    ksrc = kT if kv * HD == pr else kT2
                    kh = ksrc[pr:pr + 64, :]
                    ps_o = pso.tile([128, 512], F32, tag="o")
                    ss, pts = {}, {}

                    def emit_s(r):
                        phi = r - 4 * j
                        c0 = 128 * phi if phi > 0 else 0
                        t = psa.tile([128, 512], F32, tag="s")
                        nc.tensor.matmul(
                            t[:, c0:512], kh[:, r * 128:(r + 1) * 128],
                            qh[:, j * 512 + c0:(j + 1) * 512],
                            start=True, stop=True)
                        ss[r] = (t, c0)

                    def emit_exp(r):
                        t, c0 = ss.pop(r)
                        pt = ptp.tile([128, 512], BF16, tag="pt")
                        nc.scalar.activation(pt[:, c0:512], t[:, c0:512],
                                             AF.Exp, bias=zero_col, scale=1.0)
                        if r - 4 * j >= 0:
                            nc.vector.tensor_tensor(
                                out=pt[:, c0:c0 + 128],
                                in0=pt[:, c0:c0 + 128],
                                in1=trimask, op=ALU.mult)
                        pts[r] = (pt, c0)

                    def emit_v(r):
                        pt, c0 = pts.pop(r)
                        # columns < c0 are fully masked: skip them instead of
                        # zero-filling (they were started by earlier k-tiles)
                        nc.tensor.matmul(ps_o[0:HD + 1, c0:512],
                                         v1[kv][r], pt[:, c0:512],
                                         start=(r == 0), stop=(r == nk - 1),
                                         skip_group_check=True)

                    emit_s(0)
                    emit_exp(0)
                    if nk > 1:
                        emit_s(1)
                        emit_exp(1)
                    for r in range(2, nk):
                        emit_s(r)
                        emit_exp(r)
                        emit_v(r - 2)
                    for r in (nk - 2, nk - 1):
                        if r >= 0 and r in pts:
                            emit_v(r)

                    attn_head_tail(h, j, ps_o)

            def attn_col_pair(j, h0, h1):
                # exp over k-tile PAIRS: scores of two k-tiles land in one
                # 2-bank [128,1024] PSUM tile so one ACT Exp covers both,
                # halving ACT instruction count in the ACT-bound phase
                nk = 4 * (j + 1)
                npairs = nk // 2
                for h in range(h0, h1):
                    kv = h // (NH // NKV)
                    a_t, pr = h // 2, (h % 2) * 64
                    qh = qT[a_t][pr:pr + 64, :]
                    ksrc = kT if kv * HD == pr else kT2
                    kh = ksrc[pr:pr + 64, :]
                    ps_o = pso.tile([128, 512], F32, tag="o")
                    ss, pts = {}, {}

                    def c0_of(r):
                        phi = r - 4 * j
                        return 128 * phi if phi > 0 else 0

                    def emit_s_pair(p):
                        re, ro = 2 * p, 2 * p + 1
                        c0e, c0o = c0_of(re), c0_of(ro)
                        t = psa.tile([128, 1024], F32, tag="s")
                        nc.tensor.matmul(
                            t[:, c0e:512], kh[:, re * 128:(re + 1) * 128],
                            qh[:, j * 512 + c0e:(j + 1) * 512],
                            start=True, stop=True)
                        nc.tensor.matmul(
                            t[:, 512 + c0o:1024],
                            kh[:, ro * 128:(ro + 1) * 128],
                            qh[:, j * 512 + c0o:(j + 1) * 512],
                            start=True, stop=True)
                        ss[p] = (t, c0e, c0o)

                    def emit_exp(p):
                        t, c0e, c0o = ss.pop(p)
                        re, ro = 2 * p, 2 * p + 1
                        pt = ptp.tile([128, 1024], BF16, tag="pt")
                        # one exp spanning both halves; the dead gap
                        # [512:512+c0o) of diagonal pairs is never consumed
                        nc.scalar.activation(pt[:, c0e:1024], t[:, c0e:1024],
                                             AF.Exp, bias=zero_col, scale=1.0)
                        if re - 4 * j >= 0:
                            nc.vector.tensor_tensor(
                                out=pt[:, c0e:c0e + 128],
                                in0=pt[:, c0e:c0e + 128],
                                in1=trimask, op=ALU.mult)
                        if ro - 4 * j >= 0:
                            nc.vector.tensor_tensor(
                                out=pt[:, 512 + c0o:512 + c0o + 128],
                                in0=pt[:, 512 + c0o:512 + c0o + 128],
                                in1=trimask, op=ALU.mult)
                        pts[p] = (pt, c0e, c0o)

                    def emit_v(p):
                        pt, c0e, c0o = pts.pop(p)
                        re, ro = 2 * p, 2 * p + 1
                        nc.tensor.matmul(ps_o[0:HD + 1, c0e:512],
                                         v1[kv][re], pt[:, c0e:512],
                                         start=(re == 0), stop=False,
                                         skip_group_check=True)
                        nc.tensor.matmul(ps_o[0:HD + 1, c0o:512],
                                         v1[kv][ro], pt[:, 512 + c0o:1024],
                                         start=False, stop=(ro == nk - 1),
                                         skip_group_check=True)

                    emit_s_pair(0)
                    emit_exp(0)
                    if npairs > 1:
                        emit_s_pair(1)
                        emit_exp(1)
                    for p in range(2, npairs):
                        emit_s_pair(p)
                        emit_exp(p)
                        emit_v(p - 2)
                    for p in (npairs - 2, npairs - 1):
                        if p >= 0 and p in pts:
                            emit_v(p)

                    attn_head_tail(h, j, ps_o)

            def attn_col(j, h0=0, h1=NH):
                if exp_pair:
                    attn_col_pair(j, h0, h1)
                else:
                    attn_col_single(j, h0, h1)

            def ao_stats_pre(c):
                # normalize + local stats + stats AllGather launch; no op here
                # waits on a collective, so it can sit anywhere in the queues
                js, je = chunk_js[c]
                W = cw[c]
                CT = W // 128
                th = slice(js * 512, je * 512)
                ih = slice(js * 4, je * 4)
                sums_sb = aosc.tile([NH, W_MAX], F32, tag="sums", name="sums_sb")[:, 0:W]
                nc.sync.dma_start(out=sums_sb, in_=sums_d[:, th])
                nc.vector.reciprocal(sums_sb, sums_sb)
                nc.sync.dma_start(out=rsums_d[:, th], in_=sums_sb)
                # normalize ao in place (transposed layout, per-column rsums)
                for a in range(NO):
                    rsb = rsbp.tile([128, W_MAX], BF16, tag="rsb", name="rsb")[:, 0:W]
                    nc.gpsimd.dma_start(
                        out=rsb[0:64, :],
                        in_=rsums_d[2 * a:2 * a + 1, th].to_broadcast(
                            (64, W)))
                    nc.gpsimd.dma_start(
                        out=rsb[64:128, :],
                        in_=rsums_d[2 * a + 1:2 * a + 2, th].to_broadcast(
                            (64, W)))
                    nc.vector.tensor_tensor(out=ao[a][:, th],
                                            in0=ao[a][:, th],
                                            in1=rsb, op=ALU.mult)
                # per-token stats over the local 512 dims via PE transpose
                for i in range(js * 4, je * 4):
                    ci = slice(i, i + 1)
                    pst = pst_pool.tile([128, 512], BF16, tag="pst")
                    for a in range(NO):
                        nc.tensor.transpose(pst[:, a * 128:(a + 1) * 128],
                                            ao[a][:, i * 128:(i + 1) * 128],
                                            identity)
                    sq_scr = aosc.tile([128, 512], BF16, tag="aosq")
                    nc.scalar.activation(sq_scr, pst, AF.Square,
                                         bias=zero_col, scale=1.0,
                                         accum_out=ss_o_col[:, ci])
                    nc.vector.tensor_reduce(out=amax_o_col[:, ci], in_=pst,
                                            axis=mybir.AxisListType.X,
                                            op=ALU.max,
                                            apply_absolute_value=True)
                # pack partial stats, AllGather to [512, 2*CT]
                stats_sb = aosc.tile([128, 2 * CT_MAX], F32, tag="spack",
                                     name="spack")[:, 0:2 * CT]
                nc.vector.tensor_copy(out=stats_sb[:, 0:CT],
                                      in_=ss_o_col[:, ih])
                nc.vector.tensor_copy(out=stats_sb[:, CT:2 * CT],
                                      in_=amax_o_col[:, ih])
                nc.sync.dma_start(out=stats_in[c][:], in_=stats_sb)
                if emulate_collectives:
                    for p in range(G):
                        nc.sync.dma_start(
                            out=stats_out[c][p * 128:(p + 1) * 128, :],
                            in_=stats_in[c][:])
                else:
                    nc.gpsimd.collective_compute("AllGather", ALU.bypass,
                                                 replica_groups=rg,
                                                 ins=[stats_in[c].opt()],
                                                 outs=[stats_out[c].opt()])

            def ao_stats_post(c):
                # combine gathered stats + quant-scale math; the head of this
                # chain waits on the stats AllGather, so it is emitted half an
                # attention column after ao_stats_pre(c)
                js, je = chunk_js[c]
                W = cw[c]
                CT = W // 128
                th = slice(js * 512, je * 512)
                ih = slice(js * 4, je * 4)
                parts = [aosc.tile([128, 2 * CT_MAX], F32, tag=f"parts{p}",
                                   name=f"parts{p}")[:, 0:2 * CT]
                         for p in range(G)]
                for p in range(G):
                    nc.sync.dma_start(
                        out=parts[p],
                        in_=stats_out[c][p * 128:(p + 1) * 128, :])
                for p in range(1, G):
                    nc.vector.tensor_tensor(out=parts[0][:, 0:CT],
                                            in0=parts[0][:, 0:CT],
                                            in1=parts[p][:, 0:CT],
                                            op=ALU.add)
                    nc.vector.tensor_tensor(out=parts[0][:, CT:2 * CT],
                                            in0=parts[0][:, CT:2 * CT],
                                            in1=parts[p][:, CT:2 * CT],
                                            op=ALU.max)
                ss_full = parts[0][:, 0:CT]
                amax_full = parts[0][:, CT:2 * CT]
                rsq_o = aosc.tile([128, CT_MAX], F32, tag="rsqo", name="rsq_o")[:, 0:CT]
                xsc_o = aosc.tile([128, CT_MAX], F32, tag="xsco", name="xsc_o")[:, 0:CT]
                scr2 = aosc.tile([128, CT_MAX], F32, tag="scr2", name="scr2")[:, 0:CT]
                nc.scalar.activation(scr2, ss_full, AF.Sqrt, bias=epsn_col,
                                     scale=1.0 / (H_TOTAL * HD))
                nc.vector.reciprocal(rsq_o, scr2)
                nc.vector.tensor_tensor(out=xsc_o, in0=amax_full, in1=rsq_o,
                                        op=ALU.mult)
                nc.vector.tensor_scalar_max(xsc_o, xsc_o, EPS_Q)
                nc.vector.reciprocal(scr2, xsc_o)
                nc.vector.tensor_tensor(out=so_col[:, ih], in0=rsq_o,
                                        in1=scr2, op=ALU.mult)
                nc.vector.tensor_scalar_mul(so_col[:, ih], so_col[:, ih],
                                            127.0)
                nc.vector.tensor_scalar(fo_col[:, ih], xsc_o,
                                        ws_cols[:, 3:4], 1.0 / 127.0,
                                        op0=ALU.mult, op1=ALU.mult)
                nc.sync.dma_start(
                    out=so_d[0:1, th].rearrange("1 (i p) -> p i", p=128),
                    in_=so_col[:, ih])

            def ao_quant_gather(c):
                js, je = chunk_js[c]
                W = cw[c]
                th = slice(js * 512, je * 512)
                sob = rsbp.tile([128, W_MAX], BF16, tag="sob", name="sob")[:, 0:W]
                nc.gpsimd.dma_start(out=sob,
                                    in_=so_d[0:1, th].to_broadcast((128, W)))
                for a in range(NO):
                    tmp = qop.tile([128, W_MAX], F32, tag="qtmp", name="qtmp")[:, 0:W]
                    nc.vector.tensor_tensor(out=tmp, in0=ao[a][:, th],
                                            in1=sob, op=ALU.mult)
                    xqo = qop.tile([128, W_MAX], BF16, tag="xqo", name="xqo")[:, 0:W]
                    nc.vector.tensor_scalar(xqo, tmp, MAGIC, MAGIC,
                                            op0=ALU.add, op1=ALU.subtract)
                    xqo8 = qop.tile([128, W_MAX], I8, tag="xqo8", name="xqo8")[:, 0:W]
                    nc.vector.tensor_copy(out=xqo8, in_=xqo)
                    nc.sync.dma_start(
                        out=xqo_in[c][a * 128:(a + 1) * 128, :], in_=xqo8)
                if emulate_collectives:
                    for p in range(G):
                        nc.sync.dma_start(
                            out=xqo_out[c][p * QO:(p + 1) * QO, :],
                            in_=xqo_in[c][:])
                else:
                    nc.gpsimd.collective_compute("AllGather", ALU.bypass,
                                                 replica_groups=rg,
                                                 ins=[xqo_in[c].opt()],
                                                 outs=[xqo_out[c].opt()])

            def oproj(c):
                js, je = chunk_js[c]
                ot = [otp.tile([128, W_MAX], BF16, tag=f"ot{r}",
                               name=f"ot{r}")[:, 0:cw[c]]
                      for r in range(ND)]
                for r in range(ND):
                    # SWDGE cast-DMA int8 -> bf16, bypasses HWDGE
                    nc.gpsimd.dma_start(
                        out=ot[r], in_=xqo_out[c][r * 128:(r + 1) * 128, :])
                for il in range(cw[c] // 128):
                    i = js * 4 + il
                    ps_f = psf.tile([128, 512], F32, tag="f")
                    for r in range(ND):
                        nc.tensor.matmul(ps_f,
                                         ot[r][:, il * 128:(il + 1) * 128],
                                         wt[r][:, C_O:C_O + QO],
                                         start=(r == 0), stop=(r == ND - 1))
                    out_t = outp.tile([128, QO], F32, tag="out")
                    nc.scalar.activation(out_t, ps_f, AF.Copy, bias=0.0,
                                         scale=fo_col[:, i:i + 1])
                    nc.sync.dma_start(out=out_d[i * 128:(i + 1) * 128, :],
                                      in_=out_t)

            if driver == "orig":
                for j in range(chunk_js[0][0], chunk_js[0][1]):
                    attn_col(j)
                for c in range(NCK):
                    ao_stats_pre(c)
                    if c + 1 < NCK:
                        js, je = chunk_js[c + 1]
                        attn_col(js, 0, NH // 2)
                        ao_stats_post(c)
                        ao_quant_gather(c)
                        attn_col(js, NH // 2, NH)
                        for j in range(js + 1, je):
                            attn_col(j)
                    else:
                        # scheduling-time floors keep the gather-gated
                        # o-projections out of the attention engine queues
                        for cc in range(NCK - 1):
                            with tc.tile_wait_until(floor0 + 0.25 * cc):
                                oproj(cc)
                        ao_stats_post(c)
                        ao_quant_gather(c)
                        with tc.tile_wait_until(floor0 + 0.25 * (NCK - 1)):
                            oproj(c)
            else:
                # software pipeline over chunks: attention of chunk c runs
                # while chunk c-1 does stats/quant/gather and chunk c-2
                # o-projects. Splitting the first column's attention by
                # heads gives the stats AllGather half a column of
                # attention cover before its consumers enter the in-order
                # engine queues.
                for ci, (js, je) in enumerate(chunk_js):
                    for j in range(js, je):
                        if j == js and ci >= 1:
                            ao_stats_pre(ci - 1)
                        attn_col(j, 0, NH // 2)
                        if j == js and ci >= 1:
                            ao_stats_post(ci - 1)
                            ao_quant_gather(ci - 1)
                        if j == js and ci >= 2:
                            oproj(ci - 2)
                        attn_col(j, NH // 2, NH)
                ao_stats_pre(NCK - 1)
                if NCK >= 2:
                    # covers the last stats-AllGather with o-proj PE work
                    oproj(NCK - 2)
                ao_stats_post(NCK - 1)
                ao_quant_gather(NCK - 1)
                oproj(NCK - 1)

        es_qkv.close()  # frees qT, kT, kT2, v1
        es_ao.close()   # frees ao
        es_w.close()

    nc.compile()
    return nc


# ---------------------------------------------------------------------------
# host wrapper
# ---------------------------------------------------------------------------
_CACHE = {}


def _get_program(T, has_g):
    key = (T, has_g)
    if key not in _CACHE:
        _CACHE[key] = build_program(T=T, has_g=has_g, driver="orig",
                                    vt_v=True, exp_pair=False)
    return _CACHE[key]


def _tern(w):
    ws = max(np.mean(np.abs(w), dtype=np.float32), np.float32(EPS_Q))
    wq = np.clip(np.rint(w / ws), -1.0, 1.0).astype(np.float32)
    return wq, float(ws)


def make_in_maps(x, cos, sin, wq, wk, wv, wo, gq, gk, gv, go, T):
    import ml_dtypes
    BF = ml_dtypes.bfloat16
    cosT = np.ascontiguousarray(cos.T.astype(np.float32))      # [64, T]
    sinT = np.ascontiguousarray(sin.T.astype(np.float32))
    cost2 = np.concatenate([cosT, cosT], axis=0)               # [128, T]
    sint_signed = np.concatenate([-sinT[0:32], sinT[32:64]], axis=0)
    sint2s = np.concatenate([sint_signed, sint_signed], axis=0)
    cost2 = cost2.astype(BF)
    sint2s = sint2s.astype(BF)

    # rotate-half permutation (pure swap; sign lives in sint2s)
    pswap = np.zeros((128, 128), np.float32)
    for p in range(128):
        blk, off = (p // 64) * 64, p % 64
        pswap[p, blk + (off + 32) % 64] = 1.0
    pswap = pswap.astype(BF)

    wq_t, wsq = _tern(np.asarray(wq, np.float32))
    wk_t, wsk = _tern(np.asarray(wk, np.float32))
    wv_t, wsv = _tern(np.asarray(wv, np.float32))
    wo_t, wso = _tern(np.asarray(wo, np.float32))
    ws_r = np.array([[wsq, wsk, wsv, wso]], np.float32)

    ones = np.ones((D,), np.float32)
    has_g = not (np.array_equal(gq, ones) and np.array_equal(gk, ones)
                 and np.array_equal(gv, ones))
    if has_g:
        assert np.array_equal(gq, gk) and np.array_equal(gk, gv), \
            "per-projection norm weights must match"

    in_maps = []
    for c in range(N_CORES):
        b, g = c // G, c % G
        wcat = np.concatenate([
            wq_t[g * QO:(g + 1) * QO, :].T,
            wk_t[g * KO:(g + 1) * KO, :].T,
            wv_t[g * KO:(g + 1) * KO, :].T,
            wo_t[g * QO:(g + 1) * QO, :].T,
        ], axis=1).astype(BF)
        m = {
            "x": np.ascontiguousarray(x[b].astype(np.float32)),
            "wcat": np.ascontiguousarray(wcat),
            "cost2": cost2,
            "sint2s": sint2s,
            "pswap": pswap,
            "ws_r": ws_r,
            "go_r": np.ascontiguousarray(go[g * QO:(g + 1) * QO][None, :]),
        }
        if has_g:
            m["g_r"] = np.ascontiguousarray(gq[None, :])
        in_maps.append(m)
    return in_maps, has_g


def assemble_output(results, T):
    out = np.empty((2, T, D), np.float32)
    for c in range(N_CORES):
        b, g = c // G, c % G
        out[b][:, g * QO:(g + 1) * QO] = results[c]["out"]
    return out


def kernel(x, cos, sin, wq, wk, wv, wo, gq, gk, gv, go):
    x = np.asarray(x, np.float32)
    T = x.shape[1]
    in_maps, has_g = make_in_maps(x, cos, sin, np.asarray(wq, np.float32),
                                  np.asarray(wk, np.float32),
                                  np.asarray(wv, np.float32),
                                  np.asarray(wo, np.float32),
                                  np.asarray(gq, np.float32),
                                  np.asarray(gk, np.float32),
                                  np.asarray(gv, np.float32),
                                  np.asarray(go, np.float32), T)
    nc = _get_program(T, has_g)
    res = bass_utils.run_bass_kernel_spmd(nc, in_maps,
                                          core_ids=list(range(N_CORES)))
    return assemble_output(res.results, T)

